# revision 1
# baseline (speedup 1.0000x reference)
"""GPT block (LN -> causal MHA -> residual -> LN -> MLP -> residual) on 8 trn2 cores.

Sharding: core c = (batch b = c//2, parity o = c%2). Each core owns the
interleaved tokens o::2 of its batch. Attention rows are computed per-core for
those tokens (parity interleaving balances the causal triangle exactly and the
program is identical across cores — only input data and the partial-causal
mask differ). K/V are computed redundantly by the two cores of a batch. The
MLP is token-parallel. No cross-core communication: outputs are scattered
back on the host.

All matmuls run in bf16 (fp32 PSUM accumulation); layernorm/softmax
normalization stats stay fp32.
"""

import sys

if "/opt/trn_rl_repo" not in sys.path:
    sys.path.insert(0, "/opt/trn_rl_repo")

import numpy as np
import ml_dtypes

import concourse.bass as bass
import concourse.tile as tile
from concourse import mybir
from concourse.bass_utils import run_bass_kernel_spmd
from concourse.masks import make_identity

B, T, D, H, HD = 4, 2048, 1024, 16, 64
FF = 4 * D
P = 128
NB = T // P        # 16 key blocks
TQ = T // 2        # 1024 query tokens per core
NQ = TQ // P       # 8 query blocks per core
NC_DCH = D // P    # 8 contraction chunks over D
EPS = 1e-5
F32 = mybir.dt.float32
BF16 = mybir.dt.bfloat16
NEG = -240.0       # mask value: exp(0.125*(s-240)) ~ 1e-12, exact-enough zero
PT_TOTAL = sum(TQ - 64 * j for j in range(NB))  # 8704


def _pt_off(j):
    return 1024 * j - 32 * j * (j - 1)


def _layernorm_to_T(nc, pools, src_ap, dst, dst_col, eps_sb, id_f32, gtile, btile):
    """LN over rows of src_ap [128, D] (f32, sbuf) -> transpose -> bf16 into
    dst[:, c, dst_col:dst_col+128] for each of the 8 D-chunks."""
    lnp, pst = pools
    stats = lnp.tile([P, 2, 6], F32, tag="stats")
    for s in range(2):
        nc.vector.bn_stats(out=stats[:, s, :], in_=src_ap[:, s * 512:(s + 1) * 512])
    mv = lnp.tile([P, 2], F32, tag="mv")
    nc.vector.bn_aggr(out=mv, in_=stats)
    rstd = lnp.tile([P, 1], F32, tag="rstd")
    nc.scalar.activation(out=rstd, in_=mv[:, 1:2],
                         func=mybir.ActivationFunctionType.Sqrt,
                         bias=eps_sb, scale=1.0)
    nc.vector.reciprocal(out=rstd, in_=rstd)
    xn = lnp.tile([P, D], F32, tag="xn")
    nc.vector.tensor_scalar(out=xn, in0=src_ap, scalar1=mv[:, 0:1], scalar2=rstd,
                            op0=mybir.AluOpType.subtract, op1=mybir.AluOpType.mult)
    if gtile is not None:
        nc.vector.tensor_mul(xn, xn, gtile)
    if btile is not None:
        nc.vector.tensor_add(xn, xn, btile)
    for c in range(NC_DCH):
        tp = pst.tile([P, P], F32, tag="lntr")
        nc.tensor.transpose(tp, xn[:, c * P:(c + 1) * P], id_f32)
        nc.vector.tensor_copy(dst[:, c, dst_col:dst_col + P], tp)


def build_program(apply_g1=False, apply_g2=False):
    nc = bass.Bass()
    xb = nc.declare_dram_parameter("xb", [T, D], F32, isOutput=False)
    xq = nc.declare_dram_parameter("xq", [TQ, D], F32, isOutput=False)
    wq = nc.declare_dram_parameter("wq", [8, P, NC_DCH, P], BF16, isOutput=False)
    wk = nc.declare_dram_parameter("wk", [8, P, NC_DCH, P], BF16, isOutput=False)
    wv = nc.declare_dram_parameter("wv", [8, P, NC_DCH, P], BF16, isOutput=False)
    w1t = nc.declare_dram_parameter("w1t", [32, P, NC_DCH, P], BF16, isOutput=False)
    w2t = nc.declare_dram_parameter("w2t", [8, P, 32, P], BF16, isOutput=False)
    b1t = nc.declare_dram_parameter("b1t", [P, 32], F32, isOutput=False)
    b2t = nc.declare_dram_parameter("b2t", [P, 8], F32, isOutput=False)
    maskt = nc.declare_dram_parameter("maskt", [P, 64], F32, isOutput=False)
    gb = {}
    if apply_g1:
        gb["g1"] = nc.declare_dram_parameter("g1v", [D], F32, isOutput=False)
        gb["be1"] = nc.declare_dram_parameter("be1v", [D], F32, isOutput=False)
    if apply_g2:
        gb["g2"] = nc.declare_dram_parameter("g2v", [D], F32, isOutput=False)
        gb["be2"] = nc.declare_dram_parameter("be2v", [D], F32, isOutput=False)
    out_d = nc.declare_dram_parameter("out", [TQ, D], F32, isOutput=True)

    Exp = mybir.ActivationFunctionType.Exp
    Relu = mybir.ActivationFunctionType.Relu

    with tile.TileContext(nc) as tc:
        with tc.tile_pool(name="consts", bufs=1) as consts, \
             tc.tile_pool(name="big", bufs=1) as big:
            id_f32 = consts.tile([P, P], F32)
            make_identity(nc, id_f32)
            id_bf = consts.tile([P, P], BF16)
            make_identity(nc, id_bf)
            eps_sb = consts.tile([P, 1], F32)
            nc.vector.memset(eps_sb, EPS)
            mask_sb = consts.tile([P, 64], F32)
            nc.sync.dma_start(out=mask_sb, in_=maskt[:, :])
            b1_sb = consts.tile([P, 32], F32)
            nc.sync.dma_start(out=b1_sb, in_=b1t[:, :])
            b2_sb = consts.tile([P, 8], F32)
            nc.sync.dma_start(out=b2_sb, in_=b2t[:, :])

            def bcast(name):
                t = consts.tile([P, D], F32, tag=f"bc_{name}")
                src = gb[name]
                ap = bass.AP(tensor=src.tensor if hasattr(src, "tensor") else src[:].tensor,
                             offset=src[:].offset, ap=[[0, P]] + list(src[:].ap))
                nc.sync.dma_start(out=t, in_=ap)
                return t

            g1_t = bcast("g1") if apply_g1 else None
            be1_t = bcast("be1") if apply_g1 else None
            g2_t = bcast("g2") if apply_g2 else None
            be2_t = bcast("be2") if apply_g2 else None

            XT = big.tile([P, NC_DCH, T], BF16)    # LN1(xb)^T
            XQT = big.tile([P, NC_DCH, TQ], BF16)  # LN1(xq)^T
            xv = big.tile([P, NQ, D], F32)         # residual stream, my tokens

            # ---- Phase A: layernorm1 + transposes ----
            with tc.tile_pool(name="lnp", bufs=3) as lnp, \
                 tc.tile_pool(name="lnsrc", bufs=3) as lnsrc, \
                 tc.tile_pool(name="pst", bufs=4, space="PSUM") as pst:
                for blk in range(NB):
                    x_t = lnsrc.tile([P, D], F32, tag="xsrc")
                    nc.sync.dma_start(out=x_t, in_=xb[blk * P:(blk + 1) * P, :])
                    _layernorm_to_T(nc, (lnp, pst), x_t, XT, blk * P,
                                    eps_sb, id_f32, g1_t, be1_t)
                for kb in range(NQ):
                    nc.sync.dma_start(out=xv[:, kb, :], in_=xq[kb * P:(kb + 1) * P, :])
                    _layernorm_to_T(nc, (lnp, pst), xv[:, kb, :], XQT, kb * P,
                                    eps_sb, id_f32, g1_t, be1_t)

            # ---- Phase B/C: per head-pair projections + attention ----
            with tc.tile_pool(name="wp", bufs=2) as wp, \
                 tc.tile_pool(name="ap", bufs=2) as apool, \
                 tc.tile_pool(name="ptp", bufs=2) as ptp, \
                 tc.tile_pool(name="scr", bufs=4) as scr, \
                 tc.tile_pool(name="mm512", bufs=3, space="PSUM") as mm512, \
                 tc.tile_pool(name="tr65", bufs=2, space="PSUM") as tr65, \
                 tc.tile_pool(name="otps", bufs=2, space="PSUM") as otps:
                for pr in range(8):
                    wq_p = wp.tile([P, NC_DCH, P], BF16, tag="wq")
                    nc.sync.dma_start(out=wq_p, in_=wq[pr])
                    wk_p = wp.tile([P, NC_DCH, P], BF16, tag="wk")
                    nc.sync.dma_start(out=wk_p, in_=wk[pr])
                    wv_p = wp.tile([P, NC_DCH, P], BF16, tag="wv")
                    nc.sync.dma_start(out=wv_p, in_=wv[pr])

                    KT_p = apool.tile([P, T], BF16, tag="kt")
                    for tg in range(4):
                        ps = mm512.tile([P, 512], F32, tag="mm")
                        for c in range(NC_DCH):
                            nc.tensor.matmul(ps, lhsT=wk_p[:, c, :],
                                             rhs=XT[:, c, tg * 512:(tg + 1) * 512],
                                             start=(c == 0), stop=(c == NC_DCH - 1))
                        nc.vector.tensor_copy(KT_p[:, tg * 512:(tg + 1) * 512], ps)

                    Vaug_p = apool.tile([P, 2, NB, 65], BF16, tag="vaug")
                    nc.vector.memset(Vaug_p[:, :, :, 64:65], 1.0)
                    for tg in range(4):
                        ps = mm512.tile([P, 512], F32, tag="mm")
                        for c in range(NC_DCH):
                            nc.tensor.matmul(ps, lhsT=wv_p[:, c, :],
                                             rhs=XT[:, c, tg * 512:(tg + 1) * 512],
                                             start=(c == 0), stop=(c == NC_DCH - 1))
                        vt_sb = scr.tile([P, 512], BF16, tag="vt")
                        nc.vector.tensor_copy(vt_sb, ps)
                        for hh in range(2):
                            for s in range(4):
                                j = tg * 4 + s
                                tps = tr65.tile([P, 65], BF16, tag="tr")
                                nc.tensor.transpose(
                                    tps[:, 0:64],
                                    vt_sb[hh * 64:(hh + 1) * 64, s * P:(s + 1) * P],
                                    id_bf[hh * 64:(hh + 1) * 64, hh * 64:hh * 64 + 64])
                                nc.vector.tensor_copy(Vaug_p[:, hh, j, 0:64], tps[:, 0:64])

                    QT_p = apool.tile([P, TQ], BF16, tag="qt")
                    for tg in range(2):
                        ps = mm512.tile([P, 512], F32, tag="mm")
                        for c in range(NC_DCH):
                            nc.tensor.matmul(ps, lhsT=wq_p[:, c, :],
                                             rhs=XQT[:, c, tg * 512:(tg + 1) * 512],
                                             start=(c == 0), stop=(c == NC_DCH - 1))
                        nc.vector.tensor_copy(QT_p[:, tg * 512:(tg + 1) * 512], ps)

                    for hh in range(2):
                        h = pr * 2 + hh
                        pt = ptp.tile([P, PT_TOTAL], BF16, tag="pt")
                        hs = slice(hh * 64, (hh + 1) * 64)
                        for j in range(NB):
                            slen = TQ - 64 * j
                            off = _pt_off(j)
                            pos = 0
                            while pos < slen:
                                w = min(512, slen - pos)
                                st = mm512.tile([P, 512], F32, tag="mm")
                                nc.tensor.matmul(st[:, 0:w], lhsT=KT_p[hs, j * P:(j + 1) * P],
                                                 rhs=QT_p[hs, 64 * j + pos: 64 * j + pos + w],
                                                 start=True, stop=True)
                                if pos == 0:
                                    nc.vector.tensor_add(st[:, 0:64], st[:, 0:64], mask_sb)
                                nc.scalar.activation(out=pt[:, off + pos: off + pos + w],
                                                     in_=st[:, 0:w], func=Exp, scale=0.125)
                                pos += w
                        for kb in range(NQ):
                            ot = otps.tile([65, P], F32, tag="ot")
                            for j in range(2 * kb + 2):
                                off = _pt_off(j)
                                if j == 2 * kb + 1:
                                    nc.tensor.matmul(ot[:, 64:128],
                                                     lhsT=Vaug_p[:, hh, j, :],
                                                     rhs=pt[:, off: off + 64],
                                                     start=False, stop=True)
                                else:
                                    o = P * kb - 64 * j
                                    nc.tensor.matmul(ot, lhsT=Vaug_p[:, hh, j, :],
                                                     rhs=pt[:, off + o: off + o + P],
                                                     start=(j == 0), stop=False)
                            ot_sb = scr.tile([65, P], F32, tag="otsb")
                            nc.vector.tensor_copy(ot_sb, ot)
                            o_ps = tr65.tile([P, 65], F32, tag="tr")
                            nc.tensor.transpose(o_ps, ot_sb, id_f32[0:65, 0:65])
                            rd = scr.tile([P, 1], F32, tag="rd")
                            nc.vector.reciprocal(rd, o_ps[:, 64:65])
                            osc = scr.tile([P, 64], F32, tag="osc")
                            nc.vector.tensor_scalar_mul(osc, o_ps[:, 0:64], rd)
                            nc.vector.tensor_add(xv[:, kb, h * 64:(h + 1) * 64],
                                                 xv[:, kb, h * 64:(h + 1) * 64], osc)

            # ---- Phase D: LN2 + MLP + residual, per 512-token group ----
            with tc.tile_pool(name="x2tp", bufs=2) as x2tp, \
                 tc.tile_pool(name="h1p", bufs=1) as h1p, \
                 tc.tile_pool(name="w1s", bufs=3) as w1s, \
                 tc.tile_pool(name="w2s", bufs=2) as w2s, \
                 tc.tile_pool(name="lnp2", bufs=2) as lnp2, \
                 tc.tile_pool(name="scr2", bufs=3) as scr2, \
                 tc.tile_pool(name="mmd", bufs=3, space="PSUM") as mmd, \
                 tc.tile_pool(name="trd", bufs=2, space="PSUM") as trd:
                for g in range(2):
                    X2T = x2tp.tile([P, NC_DCH, 512], BF16, tag="x2t")
                    for s in range(4):
                        kb = g * 4 + s
                        _layernorm_to_T(nc, (lnp2, trd), xv[:, kb, :], X2T, s * P,
                                        eps_sb, id_f32, g2_t, be2_t)
                    h1 = h1p.tile([P, 32, 512], BF16, tag="h1")
                    for f in range(32):
                        w1f = w1s.tile([P, NC_DCH, P], BF16, tag="w1f")
                        nc.sync.dma_start(out=w1f, in_=w1t[f])
                        ps = mmd.tile([P, 512], F32, tag="mm")
                        for c in range(NC_DCH):
                            nc.tensor.matmul(ps, lhsT=w1f[:, c, :], rhs=X2T[:, c, :],
                                             start=(c == 0), stop=(c == NC_DCH - 1))
                        nc.scalar.activation(out=h1[:, f, :], in_=ps, func=Relu,
                                             bias=b1_sb[:, f:f + 1], scale=1.0)
                    for dd in range(8):
                        w2d = w2s.tile([P, 32, P], BF16, tag="w2d")
                        nc.sync.dma_start(out=w2d, in_=w2t[dd])
                        ps = mmd.tile([P, 512], F32, tag="mm")
                        for fc in range(32):
                            nc.tensor.matmul(ps, lhsT=w2d[:, fc, :], rhs=h1[:, fc, :],
                                             start=(fc == 0), stop=(fc == 31))
                        fsb = scr2.tile([P, 512], F32, tag="fsb")
                        nc.vector.tensor_scalar_add(fsb, ps, b2_sb[:, dd:dd + 1])
                        for s in range(4):
                            kb = g * 4 + s
                            tp = trd.tile([P, P], F32, tag="lntr")
                            nc.tensor.transpose(tp, fsb[:, s * P:(s + 1) * P], id_f32)
                            nc.vector.tensor_add(xv[:, kb, dd * P:(dd + 1) * P],
                                                 xv[:, kb, dd * P:(dd + 1) * P], tp)
                for kb in range(NQ):
                    nc.sync.dma_start(out=out_d[kb * P:(kb + 1) * P, :], in_=xv[:, kb, :])

    _split_drain_waits(nc)
    return nc


def _split_drain_waits(nc):
    """This walrus build gives every instruction a single hardware wait slot
    (one EVENTS struct per 64B instruction). Tile emits multi-wait
    instructions; move the excess waits onto single-wait NoOps inserted just
    before, on the same engine — identical semantics in program order."""
    for fn in nc.m.functions:
        for blk in fn.blocks:
            insts = blk.instructions
            i = 0
            while i < len(insts):
                inst = insts[i]
                si = inst.sync_info
                if si is not None and len(si.on_wait) > 1:
                    waits = list(si.on_wait)
                    inst.sync_info = mybir.SyncInfo(on_wait=[waits[-1]],
                                                    on_update=list(si.on_update))
                    for w in waits[:-1]:
                        nop = mybir.InstNoOp(name=nc.get_next_instruction_name(),
                                             ins=[], outs=[])
                        nop.engine = inst.engine
                        nop.sync_info = mybir.SyncInfo(on_wait=[w], on_update=[])
                        nc.register_instruction(nop, overwrite=True)
                        insts.insert(i, nop)
                        i += 1
                i += 1


def _prep_inputs(inputs, Wq, Wk, Wv, W1, b1, W2, b2, g1, be1, g2, be2,
                 apply_g1, apply_g2):
    bf = ml_dtypes.bfloat16
    f32 = np.float32
    inputs = np.ascontiguousarray(np.asarray(inputs, f32))
    wq_f = np.asarray(Wq, f32).transpose(1, 0, 2).reshape(D, D)
    wk_f = np.asarray(Wk, f32).transpose(1, 0, 2).reshape(D, D)
    wv_f = np.asarray(Wv, f32).transpose(1, 0, 2).reshape(D, D)

    def pair_tiles(w):  # [D, D] -> [8, 128, 8, 128] (pair, p, chunk, col)
        return np.ascontiguousarray(
            w.reshape(NC_DCH, P, 8, P).transpose(2, 1, 0, 3).astype(bf))

    wq_t, wk_t, wv_t = pair_tiles(wq_f), pair_tiles(wk_f), pair_tiles(wv_f)
    w1_t = np.ascontiguousarray(
        np.asarray(W1, f32).reshape(NC_DCH, P, 32, P).transpose(2, 1, 0, 3).astype(bf))
    w2_t = np.ascontiguousarray(
        np.asarray(W2, f32).reshape(32, P, 8, P).transpose(2, 1, 0, 3).astype(bf))
    b1_t = np.ascontiguousarray(np.asarray(b1, f32).reshape(32, P).T)
    b2_t = np.ascontiguousarray(np.asarray(b2, f32).reshape(8, P).T)

    in_maps = []
    for c in range(8):
        b, o = divmod(c, 2)
        xb_c = inputs[b]
        xq_c = np.ascontiguousarray(xb_c[o::2, :])
        cc, kk = np.meshgrid(np.arange(P), np.arange(64), indexing="ij")
        mask = np.where(cc <= 2 * kk + o, 0.0, NEG).astype(f32)
        m = {"xb": xb_c, "xq": xq_c, "wq": wq_t, "wk": wk_t, "wv": wv_t,
             "w1t": w1_t, "w2t": w2_t, "b1t": b1_t, "b2t": b2_t, "maskt": mask}
        if apply_g1:
            m["g1v"] = np.asarray(g1, f32)
            m["be1v"] = np.asarray(be1, f32)
        if apply_g2:
            m["g2v"] = np.asarray(g2, f32)
            m["be2v"] = np.asarray(be2, f32)
        in_maps.append(m)
    return in_maps


def _run(inputs, Wq, Wk, Wv, W1, b1, W2, b2, g1, be1, g2, be2, **spmd_kwargs):
    apply_g1 = not (np.all(np.asarray(g1) == 1.0) and np.all(np.asarray(be1) == 0.0))
    apply_g2 = not (np.all(np.asarray(g2) == 1.0) and np.all(np.asarray(be2) == 0.0))
    nc = build_program(apply_g1, apply_g2)
    in_maps = _prep_inputs(inputs, Wq, Wk, Wv, W1, b1, W2, b2, g1, be1, g2, be2,
                           apply_g1, apply_g2)
    res = run_bass_kernel_spmd(nc, in_maps, list(range(8)), **spmd_kwargs)
    out = np.empty((B, T, D), np.float32)
    for c in range(8):
        b, o = divmod(c, 2)
        out[b, o::2, :] = res.results[c]["out"]
    return out, res


def kernel(inputs, Wq, Wk, Wv, W1, b1, W2, b2, g1, be1, g2, be2):
    out, _ = _run(inputs, Wq, Wk, Wv, W1, b1, W2, b2, g1, be1, g2, be2)
    return out



# revision 13
# speedup vs baseline: 1.2856x; 1.2856x over previous
"""GPT block (LN -> causal MHA -> residual -> LN -> MLP -> residual) on 8 trn2 cores.

Sharding: core c = (batch b = c//2, parity o = c%2). Each core owns the
interleaved tokens o::2 of its batch. K/V are computed redundantly by the two
cores of a batch; attention rows and the MLP are token-parallel. No cross-core
communication: outputs are scattered back on the host.

All matmuls run in bf16 (fp32 PSUM accumulation); layernorm/softmax
normalization stats stay fp32. Structured to keep the PE array dense:
- scores for the two heads of a pair are issued interleaved at base
  partitions 0/64 so they run concurrently in disjoint row groups;
- exp runs on the scalar engine from dedicated score PSUM windows so its
  drain never blocks projection matmuls (separate PSUM pools);
- attn@V accumulates 512-query groups with causal suffix matmuls;
- all transposes are bf16 (1 cycle/col); LN normalization is applied by the
  scalar engine (Identity with per-partition scale/bias).
"""

import sys

if "/opt/trn_rl_repo" not in sys.path:
    sys.path.insert(0, "/opt/trn_rl_repo")

import numpy as np
import ml_dtypes

import concourse.bass as bass
import concourse.tile as tile
from concourse import mybir
from concourse.bass_utils import run_bass_kernel_spmd
from concourse.masks import make_identity

B, T, D, H, HD = 4, 2048, 1024, 16, 64
FF = 4 * D
P = 128
NB = T // P        # 16 key blocks
TQ = T // 2        # 1024 query tokens per core
NQ = TQ // P       # 8 query blocks per core
NC_DCH = D // P    # 8 contraction chunks over D
EPS = 1e-5
F32 = mybir.dt.float32
BF16 = mybir.dt.bfloat16
PT_TOTAL = sum(TQ - 64 * j for j in range(NB))  # 8704 = 17 * 512
NWIN = PT_TOTAL // 512  # 17 score windows per head

Exp = mybir.ActivationFunctionType.Exp
Relu = mybir.ActivationFunctionType.Relu
Sqrt = mybir.ActivationFunctionType.Sqrt
Ident = mybir.ActivationFunctionType.Identity
MUL = mybir.AluOpType.mult
ADD = mybir.AluOpType.add


def _pt_off(j):
    return 1024 * j - 32 * j * (j - 1)


def _score_chunks():
    """Flat causal score stream split at key-block and 512-col boundaries.
    Returns [(j, pos, w, off)] with off the packed pt column."""
    chunks = []
    off = 0
    for j in range(NB):
        slen = TQ - 64 * j
        pos = 0
        while pos < slen:
            w = min(512 - (off % 512), slen - pos)
            chunks.append((j, pos, w, off))
            pos += w
            off += w
    assert off == PT_TOTAL
    return chunks


def _layernorm(nc, lnp, src, dst_bf, eps_sb, gtile, btile, on_act=True):
    """LN over rows of src [P, D] f32 -> dst_bf [P, D] bf16. The normalize
    apply runs on the scalar engine (Identity(x*rstd - mu*rstd)) or on the
    vector engine (tensor_scalar) so callers can balance the two."""
    stats = lnp.tile([P, 2, 6], F32, tag="stats")
    for s in range(2):
        nc.vector.bn_stats(out=stats[:, s, :], in_=src[:, s * 512:(s + 1) * 512])
    mv = lnp.tile([P, 2], F32, tag="mv")
    nc.vector.bn_aggr(out=mv, in_=stats)
    rstd = lnp.tile([P, 1], F32, tag="rstd")
    nc.scalar.activation(out=rstd, in_=mv[:, 1:2], func=Sqrt, bias=eps_sb, scale=1.0)
    nc.vector.reciprocal(out=rstd, in_=rstd)
    if on_act:
        nmu = lnp.tile([P, 1], F32, tag="nmu")
        nc.vector.tensor_scalar(out=nmu, in0=mv[:, 0:1], scalar1=rstd, scalar2=-1.0,
                                op0=MUL, op1=MUL)
        nc.scalar.activation(out=dst_bf, in_=src, func=Ident, bias=nmu, scale=rstd)
    else:
        nc.vector.tensor_scalar(out=dst_bf, in0=src, scalar1=mv[:, 0:1],
                                scalar2=rstd, op0=mybir.AluOpType.subtract,
                                op1=MUL)
    if gtile is not None:
        nc.vector.tensor_mul(dst_bf, dst_bf, gtile)
    if btile is not None:
        nc.vector.tensor_add(dst_bf, dst_bf, btile)


def build_program(apply_g1=False, apply_g2=False):
    nc = bass.Bass()
    xb = nc.declare_dram_parameter("xb", [T, D], F32, isOutput=False)
    xq = nc.declare_dram_parameter("xq", [TQ, D], F32, isOutput=False)
    wq = nc.declare_dram_parameter("wq", [8, P, NC_DCH, P], BF16, isOutput=False)
    wk = nc.declare_dram_parameter("wk", [8, P, NC_DCH, P], BF16, isOutput=False)
    wv = nc.declare_dram_parameter("wv", [8, P, NC_DCH, P], BF16, isOutput=False)
    w1t = nc.declare_dram_parameter("w1t", [32, P, NC_DCH, P], BF16, isOutput=False)
    w2t = nc.declare_dram_parameter("w2t", [8, P, 32, P], BF16, isOutput=False)
    b1t = nc.declare_dram_parameter("b1t", [P, 32], F32, isOutput=False)
    b2t = nc.declare_dram_parameter("b2t", [P, 8], F32, isOutput=False)
    maskt = nc.declare_dram_parameter("maskt", [P, 64], BF16, isOutput=False)
    gb = {}
    if apply_g1:
        gb["g1"] = nc.declare_dram_parameter("g1v", [D], F32, isOutput=False)
        gb["be1"] = nc.declare_dram_parameter("be1v", [D], F32, isOutput=False)
    if apply_g2:
        gb["g2"] = nc.declare_dram_parameter("g2v", [D], F32, isOutput=False)
        gb["be2"] = nc.declare_dram_parameter("be2v", [D], F32, isOutput=False)
    out_d = nc.declare_dram_parameter("out", [TQ, D], F32, isOutput=True)

    chunks = _score_chunks()
    win_chunks = [[] for _ in range(NWIN)]
    for c in chunks:
        win_chunks[c[3] // 512].append(c)
    win_masks = [[] for _ in range(NWIN)]
    for j in range(NB):
        off = _pt_off(j)
        win_masks[off // 512].append(off)

    with tile.TileContext(nc) as tc:
        with tc.tile_pool(name="consts", bufs=1) as consts, \
             tc.tile_pool(name="big", bufs=1) as big:
            id_bf = consts.tile([P, P], BF16)
            make_identity(nc, id_bf)
            eps_sb = consts.tile([P, 1], F32)
            nc.vector.memset(eps_sb, EPS)
            mask_sb = consts.tile([P, 64], BF16)
            nc.sync.dma_start(out=mask_sb, in_=maskt[:, :])
            b1_sb = consts.tile([P, 32], F32)
            nc.sync.dma_start(out=b1_sb, in_=b1t[:, :])
            b2_sb = consts.tile([P, 8], F32)
            nc.sync.dma_start(out=b2_sb, in_=b2t[:, :])

            def bcast(name):
                t = consts.tile([P, D], F32, tag=f"bc_{name}")
                src = gb[name]
                ap = bass.AP(tensor=src.tensor if hasattr(src, "tensor") else src[:].tensor,
                             offset=src[:].offset, ap=[[0, P]] + list(src[:].ap))
                nc.sync.dma_start(out=t, in_=ap)
                return t

            g1_t = bcast("g1") if apply_g1 else None
            be1_t = bcast("be1") if apply_g1 else None
            g2_t = bcast("g2") if apply_g2 else None
            be2_t = bcast("be2") if apply_g2 else None

            XT = big.tile([P, NC_DCH, T], BF16)    # LN1(xb)^T
            XQT = big.tile([P, NC_DCH, TQ], BF16)  # strided query columns of XT
            xv = big.tile([P, NQ, D], F32)         # residual stream, my tokens

            # ---- Phase A: layernorm1 + transposes ----
            with tc.tile_pool(name="lnp", bufs=3) as lnp, \
                 tc.tile_pool(name="lnsrc", bufs=3) as lnsrc, \
                 tc.tile_pool(name="trp", bufs=1, space="PSUM") as trp:
                for blk in range(NB):
                    x_t = lnsrc.tile([P, D], F32, tag="xsrc")
                    nc.sync.dma_start(out=x_t, in_=xb[blk * P:(blk + 1) * P, :])
                    xn = lnp.tile([P, D], BF16, tag="xn")
                    _layernorm(nc, lnp, x_t, xn, eps_sb, g1_t, be1_t,
                               on_act=(blk % 2 == 0))
                    lt = trp.tile([P, NC_DCH, P], BF16, tag="lntr")
                    for c in range(NC_DCH):
                        nc.tensor.matmul(lt[:, c, :], lhsT=xn[:, c * P:(c + 1) * P],
                                         rhs=id_bf, is_transpose=True,
                                         start=(c == 0), stop=(c == NC_DCH - 1),
                                         skip_group_check=True)
                    nc.vector.tensor_copy(XT[:, :, blk * P:(blk + 1) * P], lt)
                for kb in range(NQ):
                    nc.sync.dma_start(out=xv[:, kb, :], in_=xq[kb * P:(kb + 1) * P, :])
                    xnq = lnp.tile([P, D], BF16, tag="xn")
                    _layernorm(nc, lnp, xv[:, kb, :], xnq, eps_sb, g1_t, be1_t,
                               on_act=(kb % 2 == 0))
                    ltq = trp.tile([P, NC_DCH, P], BF16, tag="lntr")
                    for c in range(NC_DCH):
                        nc.tensor.matmul(ltq[:, c, :], lhsT=xnq[:, c * P:(c + 1) * P],
                                         rhs=id_bf, is_transpose=True,
                                         start=(c == 0), stop=(c == NC_DCH - 1),
                                         skip_group_check=True)
                    nc.vector.tensor_copy(XQT[:, :, kb * P:(kb + 1) * P], ltq)

            # ---- Phase B/C: per head-pair projections + attention ----
            with tc.tile_pool(name="wp", bufs=2) as wp, \
                 tc.tile_pool(name="ap", bufs=2) as apool, \
                 tc.tile_pool(name="ptp", bufs=1) as ptp, \
                 tc.tile_pool(name="scr", bufs=3) as scr, \
                 tc.tile_pool(name="pp", bufs=2, space="PSUM") as pp, \
                 tc.tile_pool(name="sw", bufs=2, space="PSUM") as sw, \
                 tc.tile_pool(name="avp", bufs=1, space="PSUM") as avp, \
                 tc.tile_pool(name="trb", bufs=2, space="PSUM") as trb:
                for pr in range(8):
                    wq_p = wp.tile([P, NC_DCH, P], BF16, tag="wq")
                    nc.sync.dma_start(out=wq_p, in_=wq[pr])
                    wk_p = wp.tile([P, NC_DCH, P], BF16, tag="wk")
                    nc.sync.dma_start(out=wk_p, in_=wk[pr])
                    wv_p = wp.tile([P, NC_DCH, P], BF16, tag="wv")
                    nc.sync.dma_start(out=wv_p, in_=wv[pr])

                    KT_p = apool.tile([P, T], BF16, tag="kt")
                    for tg in range(4):
                        ps = pp.tile([P, 512], F32, tag="mm")
                        for c in range(NC_DCH):
                            nc.tensor.matmul(ps, lhsT=wk_p[:, c, :],
                                             rhs=XT[:, c, tg * 512:(tg + 1) * 512],
                                             start=(c == 0), stop=(c == NC_DCH - 1))
                        nc.vector.tensor_copy(KT_p[:, tg * 512:(tg + 1) * 512], ps)

                    Vaug_p = apool.tile([P, 2, NB, 65], BF16, tag="vaug")
                    nc.vector.memset(Vaug_p[:, :, :, 64:65], 1.0)
                    for tg in range(4):
                        ps = pp.tile([P, 512], F32, tag="mm")
                        for c in range(NC_DCH):
                            nc.tensor.matmul(ps, lhsT=wv_p[:, c, :],
                                             rhs=XT[:, c, tg * 512:(tg + 1) * 512],
                                             start=(c == 0), stop=(c == NC_DCH - 1))
                        vt_sb = scr.tile([P, 512], BF16, tag="vt")
                        nc.vector.tensor_copy(vt_sb, ps)
                        for hh in range(2):
                            # one accumulation group per bank, uniform base
                            # partition within the group (mixed bases wedge
                            # the PE)
                            tb = trb.tile([P, 4, 64], BF16, tag="tr")
                            for s in range(4):
                                nc.tensor.matmul(
                                    tb[:, s, :],
                                    lhsT=vt_sb[hh * 64:(hh + 1) * 64, s * P:(s + 1) * P],
                                    rhs=id_bf[hh * 64:(hh + 1) * 64, hh * 64:hh * 64 + 64],
                                    is_transpose=True,
                                    start=(s == 0), stop=(s == 3),
                                    skip_group_check=True)
                            nc.vector.tensor_copy(
                                Vaug_p[:, hh, tg * 4:(tg + 1) * 4, 0:64], tb)

                    QT_p = apool.tile([P, TQ], BF16, tag="qt")
                    for tg in range(2):
                        ps = pp.tile([P, 512], F32, tag="mm")
                        for c in range(NC_DCH):
                            nc.tensor.matmul(ps, lhsT=wq_p[:, c, :],
                                             rhs=XQT[:, c, tg * 512:(tg + 1) * 512],
                                             start=(c == 0), stop=(c == NC_DCH - 1))
                        nc.vector.tensor_copy(QT_p[:, tg * 512:(tg + 1) * 512], ps)

                    # scores: both heads interleaved (row groups 0-63 / 64-127)
                    pt0 = ptp.tile([P, PT_TOTAL], BF16, tag="pt0")
                    pt1 = ptp.tile([P, PT_TOTAL], BF16, tag="pt1")
                    pts = [pt0, pt1]
                    for w in range(NWIN):
                        win0 = sw.tile([P, 512], F32, tag="sw")
                        win1 = sw.tile([P, 512], F32, tag="sw")
                        wins = [win0, win1]
                        ncw = len(win_chunks[w])
                        for ci, (j, pos, wd, off) in enumerate(win_chunks[w]):
                            wcol = off % 512
                            for hh in range(2):
                                hs = slice(hh * 64, (hh + 1) * 64)
                                nc.tensor.matmul(
                                    wins[hh][:, wcol:wcol + wd],
                                    lhsT=KT_p[hs, j * P:(j + 1) * P],
                                    rhs=QT_p[hs, 64 * j + pos: 64 * j + pos + wd],
                                    start=(ci == 0), stop=(ci == ncw - 1),
                                    skip_group_check=True)
                        for hh in range(2):
                            nc.scalar.activation(
                                out=pts[hh][:, w * 512:(w + 1) * 512],
                                in_=wins[hh], func=Exp, scale=0.125)
                        for off in win_masks[w]:
                            for hh in range(2):
                                nc.vector.tensor_mul(pts[hh][:, off:off + 64],
                                                     pts[hh][:, off:off + 64],
                                                     mask_sb)

                    # attn @ V: 512-query groups, causal suffix accumulation
                    for hh in range(2):
                        h = pr * 2 + hh
                        pt = pts[hh]
                        for g in range(2):
                            ot = avp.tile([65, 512], F32, tag="av")
                            jmax = 8 * (g + 1)
                            for j in range(jmax):
                                qlo = max(512 * g, 64 * j)
                                wd = 512 * (g + 1) - qlo
                                nc.tensor.matmul(
                                    ot[:, qlo - 512 * g: 512],
                                    lhsT=Vaug_p[:, hh, j, :],
                                    rhs=pt[:, _pt_off(j) + qlo - 64 * j:
                                           _pt_off(j) + qlo - 64 * j + wd],
                                    start=(j == 0), stop=(j == jmax - 1))
                            ob = scr.tile([65, 512], BF16, tag="ob")
                            nc.vector.tensor_copy(ob, ot)
                            tp = trb.tile([P, 4, 96], BF16, tag="tr")
                            for s in range(4):
                                nc.tensor.matmul(tp[:, s, 0:65],
                                                 lhsT=ob[:, s * P:(s + 1) * P],
                                                 rhs=id_bf[0:65, 0:65],
                                                 is_transpose=True,
                                                 start=(s == 0), stop=(s == 3),
                                                 skip_group_check=True)
                            rd = scr.tile([P, 4], F32, tag="rd")
                            nc.vector.reciprocal(rd, tp[:, :, 64])
                            for s in range(4):
                                kb = 4 * g + s
                                nc.vector.scalar_tensor_tensor(
                                    out=xv[:, kb, h * 64:(h + 1) * 64],
                                    in0=tp[:, s, 0:64],
                                    scalar=rd[:, s:s + 1],
                                    in1=xv[:, kb, h * 64:(h + 1) * 64],
                                    op0=MUL, op1=ADD)

            # ---- Phase D: LN2 + MLP + residual over all 1024 tokens ----
            with tc.tile_pool(name="x2tp", bufs=1) as x2tp, \
                 tc.tile_pool(name="h1p", bufs=1) as h1p, \
                 tc.tile_pool(name="w1s", bufs=3) as w1s, \
                 tc.tile_pool(name="w2s", bufs=2) as w2s, \
                 tc.tile_pool(name="lnp2", bufs=2) as lnp2, \
                 tc.tile_pool(name="scr2", bufs=3) as scr2, \
                 tc.tile_pool(name="mmd", bufs=2, space="PSUM") as mmd, \
                 tc.tile_pool(name="trd", bufs=2, space="PSUM") as trd:
                X2T = x2tp.tile([P, NC_DCH, TQ], BF16, tag="x2t")
                for kb in range(NQ):
                    xn2 = lnp2.tile([P, D], BF16, tag="xn2")
                    _layernorm(nc, lnp2, xv[:, kb, :], xn2, eps_sb, g2_t, be2_t)
                    lt = trd.tile([P, NC_DCH, P], BF16, tag="lntr")
                    for c in range(NC_DCH):
                        nc.tensor.matmul(lt[:, c, :], lhsT=xn2[:, c * P:(c + 1) * P],
                                         rhs=id_bf, is_transpose=True,
                                         start=(c == 0), stop=(c == NC_DCH - 1),
                                         skip_group_check=True)
                    nc.vector.tensor_copy(X2T[:, :, kb * P:(kb + 1) * P], lt)
                h1 = h1p.tile([P, 32, TQ], BF16, tag="h1")
                for f in range(32):
                    w1f = w1s.tile([P, NC_DCH, P], BF16, tag="w1f")
                    nc.sync.dma_start(out=w1f, in_=w1t[f])
                    ps = mmd.tile([P, TQ], F32, tag="mm")
                    for g in range(2):
                        for c in range(NC_DCH):
                            nc.tensor.matmul(ps[:, g * 512:(g + 1) * 512],
                                             lhsT=w1f[:, c, :],
                                             rhs=X2T[:, c, g * 512:(g + 1) * 512],
                                             start=(c == 0), stop=(c == NC_DCH - 1))
                    nc.scalar.activation(out=h1[:, f, :], in_=ps, func=Relu,
                                         bias=b1_sb[:, f:f + 1], scale=1.0)
                for dd in range(8):
                    w2d = w2s.tile([P, 32, P], BF16, tag="w2d")
                    nc.sync.dma_start(out=w2d, in_=w2t[dd])
                    ps = mmd.tile([P, TQ], F32, tag="mm")
                    for g in range(2):
                        for fc in range(32):
                            nc.tensor.matmul(ps[:, g * 512:(g + 1) * 512],
                                             lhsT=w2d[:, fc, :],
                                             rhs=h1[:, fc, g * 512:(g + 1) * 512],
                                             start=(fc == 0), stop=(fc == 31))
                    fsb = scr2.tile([P, TQ], BF16, tag="fsb")
                    nc.vector.tensor_scalar_add(fsb, ps, b2_sb[:, dd:dd + 1])
                    ft = trd.tile([P, NQ, P], BF16, tag="fftr")
                    for kb in range(NQ):
                        nc.tensor.matmul(ft[:, kb, :],
                                         lhsT=fsb[:, kb * P:(kb + 1) * P],
                                         rhs=id_bf, is_transpose=True,
                                         start=(kb == 0), stop=(kb == NQ - 1),
                                         skip_group_check=True)
                    nc.vector.tensor_add(xv[:, :, dd * P:(dd + 1) * P],
                                         xv[:, :, dd * P:(dd + 1) * P], ft)
                for kb in range(NQ):
                    nc.sync.dma_start(out=out_d[kb * P:(kb + 1) * P, :], in_=xv[:, kb, :])

    _split_drain_waits(nc)
    return nc


def _split_drain_waits(nc):
    """This walrus build gives every instruction a single hardware wait slot
    (one EVENTS struct per 64B instruction). Tile emits multi-wait
    instructions; move the excess waits onto single-wait NoOps inserted just
    before, on the same engine — identical semantics in program order."""
    for fn in nc.m.functions:
        for blk in fn.blocks:
            insts = blk.instructions
            i = 0
            while i < len(insts):
                inst = insts[i]
                si = inst.sync_info
                if si is not None and len(si.on_wait) > 1:
                    waits = list(si.on_wait)
                    inst.sync_info = mybir.SyncInfo(on_wait=[waits[-1]],
                                                    on_update=list(si.on_update))
                    for w in waits[:-1]:
                        nop = mybir.InstNoOp(name=nc.get_next_instruction_name(),
                                             ins=[], outs=[])
                        nop.engine = inst.engine
                        nop.sync_info = mybir.SyncInfo(on_wait=[w], on_update=[])
                        nc.register_instruction(nop, overwrite=True)
                        insts.insert(i, nop)
                        i += 1
                i += 1


def _prep_inputs(inputs, Wq, Wk, Wv, W1, b1, W2, b2, g1, be1, g2, be2,
                 apply_g1, apply_g2):
    bf = ml_dtypes.bfloat16
    f32 = np.float32
    inputs = np.ascontiguousarray(np.asarray(inputs, f32))
    wq_f = np.asarray(Wq, f32).transpose(1, 0, 2).reshape(D, D)
    wk_f = np.asarray(Wk, f32).transpose(1, 0, 2).reshape(D, D)
    wv_f = np.asarray(Wv, f32).transpose(1, 0, 2).reshape(D, D)

    def pair_tiles(w):  # [D, D] -> [8, 128, 8, 128] (pair, p, chunk, col)
        return np.ascontiguousarray(
            w.reshape(NC_DCH, P, 8, P).transpose(2, 1, 0, 3).astype(bf))

    wq_t, wk_t, wv_t = pair_tiles(wq_f), pair_tiles(wk_f), pair_tiles(wv_f)
    w1_t = np.ascontiguousarray(
        np.asarray(W1, f32).reshape(NC_DCH, P, 32, P).transpose(2, 1, 0, 3).astype(bf))
    w2_t = np.ascontiguousarray(
        np.asarray(W2, f32).reshape(32, P, 8, P).transpose(2, 1, 0, 3).astype(bf))
    b1_t = np.ascontiguousarray(np.asarray(b1, f32).reshape(32, P).T)
    b2_t = np.ascontiguousarray(np.asarray(b2, f32).reshape(8, P).T)

    cc, kk = np.meshgrid(np.arange(P), np.arange(64), indexing="ij")
    in_maps = []
    for c in range(8):
        b, o = divmod(c, 2)
        xb_c = inputs[b]
        xq_c = np.ascontiguousarray(xb_c[o::2, :])
        mask = np.ascontiguousarray((cc <= 2 * kk + o).astype(f32).astype(bf))
        m = {"xb": xb_c, "xq": xq_c, "wq": wq_t, "wk": wk_t, "wv": wv_t,
             "w1t": w1_t, "w2t": w2_t, "b1t": b1_t, "b2t": b2_t, "maskt": mask}
        if apply_g1:
            m["g1v"] = np.asarray(g1, f32)
            m["be1v"] = np.asarray(be1, f32)
        if apply_g2:
            m["g2v"] = np.asarray(g2, f32)
            m["be2v"] = np.asarray(be2, f32)
        in_maps.append(m)
    return in_maps


def _run(inputs, Wq, Wk, Wv, W1, b1, W2, b2, g1, be1, g2, be2, **spmd_kwargs):
    apply_g1 = not (np.all(np.asarray(g1) == 1.0) and np.all(np.asarray(be1) == 0.0))
    apply_g2 = not (np.all(np.asarray(g2) == 1.0) and np.all(np.asarray(be2) == 0.0))
    nc = build_program(apply_g1, apply_g2)
    in_maps = _prep_inputs(inputs, Wq, Wk, Wv, W1, b1, W2, b2, g1, be1, g2, be2,
                           apply_g1, apply_g2)
    res = run_bass_kernel_spmd(nc, in_maps, list(range(8)), **spmd_kwargs)
    out = np.empty((B, T, D), np.float32)
    for c in range(8):
        b, o = divmod(c, 2)
        out[b, o::2, :] = res.results[c]["out"]
    return out, res


def kernel(inputs, Wq, Wk, Wv, W1, b1, W2, b2, g1, be1, g2, be2):
    out, _ = _run(inputs, Wq, Wk, Wv, W1, b1, W2, b2, g1, be1, g2, be2)
    return out


# revision 14
# speedup vs baseline: 1.2952x; 1.0075x over previous
"""GPT block (LN -> causal MHA -> residual -> LN -> MLP -> residual) on 8 trn2 cores.

Sharding: core c = (batch b = c//2, parity o = c%2). Each core owns the
interleaved tokens o::2 of its batch. K/V are computed redundantly by the two
cores of a batch; attention rows and the MLP are token-parallel. No cross-core
communication: outputs are scattered back on the host.

All matmuls run in bf16 (fp32 PSUM accumulation); layernorm/softmax
normalization stats stay fp32. Structured to keep the PE array dense:
- scores for the two heads of a pair are issued interleaved at base
  partitions 0/64 so they run concurrently in disjoint row groups;
- exp runs on the scalar engine from dedicated score PSUM windows so its
  drain never blocks projection matmuls (separate PSUM pools);
- attn@V accumulates 512-query groups with causal suffix matmuls;
- all transposes are bf16 (1 cycle/col); LN normalization is applied by the
  scalar engine (Identity with per-partition scale/bias).
"""

import sys

if "/opt/trn_rl_repo" not in sys.path:
    sys.path.insert(0, "/opt/trn_rl_repo")

import numpy as np
import ml_dtypes

import concourse.bass as bass
import concourse.tile as tile
from concourse import mybir
from concourse.bass_utils import run_bass_kernel_spmd
from concourse.masks import make_identity

B, T, D, H, HD = 4, 2048, 1024, 16, 64
FF = 4 * D
P = 128
NB = T // P        # 16 key blocks
TQ = T // 2        # 1024 query tokens per core
NQ = TQ // P       # 8 query blocks per core
NC_DCH = D // P    # 8 contraction chunks over D
EPS = 1e-5
F32 = mybir.dt.float32
BF16 = mybir.dt.bfloat16
PT_TOTAL = sum(TQ - 64 * j for j in range(NB))  # 8704 = 17 * 512
NWIN = PT_TOTAL // 512  # 17 score windows per head

Exp = mybir.ActivationFunctionType.Exp
Relu = mybir.ActivationFunctionType.Relu
Sqrt = mybir.ActivationFunctionType.Sqrt
Ident = mybir.ActivationFunctionType.Identity
MUL = mybir.AluOpType.mult
ADD = mybir.AluOpType.add


def _pt_off(j):
    return 1024 * j - 32 * j * (j - 1)


def _score_chunks():
    """Flat causal score stream split at key-block and 512-col boundaries.
    Returns [(j, pos, w, off)] with off the packed pt column."""
    chunks = []
    off = 0
    for j in range(NB):
        slen = TQ - 64 * j
        pos = 0
        while pos < slen:
            w = min(512 - (off % 512), slen - pos)
            chunks.append((j, pos, w, off))
            pos += w
            off += w
    assert off == PT_TOTAL
    return chunks


def _layernorm(nc, lnp, src, dst_bf, eps_sb, gtile, btile, on_act=True):
    """LN over rows of src [P, D] f32 -> dst_bf [P, D] bf16. The normalize
    apply runs on the scalar engine (Identity(x*rstd - mu*rstd)) or on the
    vector engine (tensor_scalar) so callers can balance the two."""
    stats = lnp.tile([P, 2, 6], F32, tag="stats")
    for s in range(2):
        nc.vector.bn_stats(out=stats[:, s, :], in_=src[:, s * 512:(s + 1) * 512])
    mv = lnp.tile([P, 2], F32, tag="mv")
    nc.vector.bn_aggr(out=mv, in_=stats)
    rstd = lnp.tile([P, 1], F32, tag="rstd")
    nc.scalar.activation(out=rstd, in_=mv[:, 1:2], func=Sqrt, bias=eps_sb, scale=1.0)
    nc.vector.reciprocal(out=rstd, in_=rstd)
    if on_act:
        nmu = lnp.tile([P, 1], F32, tag="nmu")
        nc.vector.tensor_scalar(out=nmu, in0=mv[:, 0:1], scalar1=rstd, scalar2=-1.0,
                                op0=MUL, op1=MUL)
        nc.scalar.activation(out=dst_bf, in_=src, func=Ident, bias=nmu, scale=rstd)
    else:
        nc.vector.tensor_scalar(out=dst_bf, in0=src, scalar1=mv[:, 0:1],
                                scalar2=rstd, op0=mybir.AluOpType.subtract,
                                op1=MUL)
    if gtile is not None:
        nc.vector.tensor_mul(dst_bf, dst_bf, gtile)
    if btile is not None:
        nc.vector.tensor_add(dst_bf, dst_bf, btile)


def build_program(apply_g1=False, apply_g2=False):
    nc = bass.Bass()
    xb = nc.declare_dram_parameter("xb", [T, D], F32, isOutput=False)
    xq = nc.declare_dram_parameter("xq", [TQ, D], F32, isOutput=False)
    wq = nc.declare_dram_parameter("wq", [8, P, NC_DCH, P], BF16, isOutput=False)
    wk = nc.declare_dram_parameter("wk", [8, P, NC_DCH, P], BF16, isOutput=False)
    wv = nc.declare_dram_parameter("wv", [8, P, NC_DCH, P], BF16, isOutput=False)
    w1t = nc.declare_dram_parameter("w1t", [32, P, NC_DCH, P], BF16, isOutput=False)
    w2t = nc.declare_dram_parameter("w2t", [8, P, 32, P], BF16, isOutput=False)
    b1t = nc.declare_dram_parameter("b1t", [P, 32], F32, isOutput=False)
    b2t = nc.declare_dram_parameter("b2t", [P, 8], F32, isOutput=False)
    maskt = nc.declare_dram_parameter("maskt", [P, 64], BF16, isOutput=False)
    gb = {}
    if apply_g1:
        gb["g1"] = nc.declare_dram_parameter("g1v", [D], F32, isOutput=False)
        gb["be1"] = nc.declare_dram_parameter("be1v", [D], F32, isOutput=False)
    if apply_g2:
        gb["g2"] = nc.declare_dram_parameter("g2v", [D], F32, isOutput=False)
        gb["be2"] = nc.declare_dram_parameter("be2v", [D], F32, isOutput=False)
    out_d = nc.declare_dram_parameter("out", [TQ, D], F32, isOutput=True)

    chunks = _score_chunks()
    win_chunks = [[] for _ in range(NWIN)]
    for c in chunks:
        win_chunks[c[3] // 512].append(c)
    win_masks = [[] for _ in range(NWIN)]
    for j in range(NB):
        off = _pt_off(j)
        win_masks[off // 512].append(off)

    with tile.TileContext(nc) as tc:
        with tc.tile_pool(name="consts", bufs=1) as consts, \
             tc.tile_pool(name="big", bufs=1) as big:
            id_bf = consts.tile([P, P], BF16)
            make_identity(nc, id_bf)
            eps_sb = consts.tile([P, 1], F32)
            nc.vector.memset(eps_sb, EPS)
            mask_sb = consts.tile([P, 64], BF16)
            nc.sync.dma_start(out=mask_sb, in_=maskt[:, :])
            b1_sb = consts.tile([P, 32], F32)
            nc.sync.dma_start(out=b1_sb, in_=b1t[:, :])
            b2_sb = consts.tile([P, 8], F32)
            nc.sync.dma_start(out=b2_sb, in_=b2t[:, :])

            def bcast(name):
                t = consts.tile([P, D], F32, tag=f"bc_{name}")
                src = gb[name]
                ap = bass.AP(tensor=src.tensor if hasattr(src, "tensor") else src[:].tensor,
                             offset=src[:].offset, ap=[[0, P]] + list(src[:].ap))
                nc.sync.dma_start(out=t, in_=ap)
                return t

            g1_t = bcast("g1") if apply_g1 else None
            be1_t = bcast("be1") if apply_g1 else None
            g2_t = bcast("g2") if apply_g2 else None
            be2_t = bcast("be2") if apply_g2 else None

            XT = big.tile([P, NC_DCH, T], BF16)    # LN1(xb)^T
            XQT = big.tile([P, NC_DCH, TQ], BF16)  # strided query columns of XT
            xv = big.tile([P, NQ, D], F32)         # residual stream, my tokens

            # ---- Phase A: layernorm1 + transposes ----
            with tc.tile_pool(name="lnp", bufs=4) as lnp, \
                 tc.tile_pool(name="lnsrc", bufs=4) as lnsrc, \
                 tc.tile_pool(name="trp", bufs=1, space="PSUM") as trp:
                for blk in range(NB):
                    x_t = lnsrc.tile([P, D], F32, tag="xsrc")
                    nc.sync.dma_start(out=x_t, in_=xb[blk * P:(blk + 1) * P, :])
                    xn = lnp.tile([P, D], BF16, tag="xn")
                    _layernorm(nc, lnp, x_t, xn, eps_sb, g1_t, be1_t,
                               on_act=(blk % 2 == 0))
                    lt = trp.tile([P, NC_DCH, P], BF16, tag="lntr")
                    for c in range(NC_DCH):
                        nc.tensor.matmul(lt[:, c, :], lhsT=xn[:, c * P:(c + 1) * P],
                                         rhs=id_bf, is_transpose=True,
                                         start=(c == 0), stop=(c == NC_DCH - 1),
                                         skip_group_check=True)
                    nc.vector.tensor_copy(XT[:, :, blk * P:(blk + 1) * P], lt)
                for kb in range(NQ):
                    nc.sync.dma_start(out=xv[:, kb, :], in_=xq[kb * P:(kb + 1) * P, :])
                    xnq = lnp.tile([P, D], BF16, tag="xn")
                    _layernorm(nc, lnp, xv[:, kb, :], xnq, eps_sb, g1_t, be1_t,
                               on_act=(kb % 2 == 0))
                    ltq = trp.tile([P, NC_DCH, P], BF16, tag="lntr")
                    for c in range(NC_DCH):
                        nc.tensor.matmul(ltq[:, c, :], lhsT=xnq[:, c * P:(c + 1) * P],
                                         rhs=id_bf, is_transpose=True,
                                         start=(c == 0), stop=(c == NC_DCH - 1),
                                         skip_group_check=True)
                    nc.vector.tensor_copy(XQT[:, :, kb * P:(kb + 1) * P], ltq)

            # ---- Phase B/C: per head-pair projections + attention ----
            with tc.tile_pool(name="wp", bufs=2) as wp, \
                 tc.tile_pool(name="ap", bufs=2) as apool, \
                 tc.tile_pool(name="ptp", bufs=1) as ptp, \
                 tc.tile_pool(name="scr", bufs=4) as scr, \
                 tc.tile_pool(name="pp", bufs=2, space="PSUM") as pp, \
                 tc.tile_pool(name="sw", bufs=2, space="PSUM") as sw, \
                 tc.tile_pool(name="avp", bufs=1, space="PSUM") as avp, \
                 tc.tile_pool(name="trb", bufs=2, space="PSUM") as trb:
                for pr in range(8):
                    wq_p = wp.tile([P, NC_DCH, P], BF16, tag="wq")
                    nc.sync.dma_start(out=wq_p, in_=wq[pr])
                    wk_p = wp.tile([P, NC_DCH, P], BF16, tag="wk")
                    nc.sync.dma_start(out=wk_p, in_=wk[pr])
                    wv_p = wp.tile([P, NC_DCH, P], BF16, tag="wv")
                    nc.sync.dma_start(out=wv_p, in_=wv[pr])

                    KT_p = apool.tile([P, T], BF16, tag="kt")
                    for tg in range(4):
                        ps = pp.tile([P, 512], F32, tag="mm")
                        for c in range(NC_DCH):
                            nc.tensor.matmul(ps, lhsT=wk_p[:, c, :],
                                             rhs=XT[:, c, tg * 512:(tg + 1) * 512],
                                             start=(c == 0), stop=(c == NC_DCH - 1))
                        nc.vector.tensor_copy(KT_p[:, tg * 512:(tg + 1) * 512], ps)

                    Vaug_p = apool.tile([P, 2, NB, 65], BF16, tag="vaug")
                    nc.vector.memset(Vaug_p[:, :, :, 64:65], 1.0)
                    for tg in range(4):
                        ps = pp.tile([P, 512], F32, tag="mm")
                        for c in range(NC_DCH):
                            nc.tensor.matmul(ps, lhsT=wv_p[:, c, :],
                                             rhs=XT[:, c, tg * 512:(tg + 1) * 512],
                                             start=(c == 0), stop=(c == NC_DCH - 1))
                        vt_sb = scr.tile([P, 512], BF16, tag="vt")
                        nc.vector.tensor_copy(vt_sb, ps)
                        for hh in range(2):
                            # one accumulation group per bank, uniform base
                            # partition within the group (mixed bases wedge
                            # the PE)
                            tb = trb.tile([P, 4, 64], BF16, tag="tr")
                            for s in range(4):
                                nc.tensor.matmul(
                                    tb[:, s, :],
                                    lhsT=vt_sb[hh * 64:(hh + 1) * 64, s * P:(s + 1) * P],
                                    rhs=id_bf[hh * 64:(hh + 1) * 64, hh * 64:hh * 64 + 64],
                                    is_transpose=True,
                                    start=(s == 0), stop=(s == 3),
                                    skip_group_check=True)
                            nc.vector.tensor_copy(
                                Vaug_p[:, hh, tg * 4:(tg + 1) * 4, 0:64], tb)

                    QT_p = apool.tile([P, TQ], BF16, tag="qt")
                    for tg in range(2):
                        ps = pp.tile([P, 512], F32, tag="mm")
                        for c in range(NC_DCH):
                            nc.tensor.matmul(ps, lhsT=wq_p[:, c, :],
                                             rhs=XQT[:, c, tg * 512:(tg + 1) * 512],
                                             start=(c == 0), stop=(c == NC_DCH - 1))
                        nc.vector.tensor_copy(QT_p[:, tg * 512:(tg + 1) * 512], ps)

                    # scores: both heads interleaved (row groups 0-63 / 64-127)
                    pt0 = ptp.tile([P, PT_TOTAL], BF16, tag="pt0")
                    pt1 = ptp.tile([P, PT_TOTAL], BF16, tag="pt1")
                    pts = [pt0, pt1]
                    for w in range(NWIN):
                        win0 = sw.tile([P, 512], F32, tag="sw")
                        win1 = sw.tile([P, 512], F32, tag="sw")
                        wins = [win0, win1]
                        ncw = len(win_chunks[w])
                        for ci, (j, pos, wd, off) in enumerate(win_chunks[w]):
                            wcol = off % 512
                            for hh in range(2):
                                hs = slice(hh * 64, (hh + 1) * 64)
                                nc.tensor.matmul(
                                    wins[hh][:, wcol:wcol + wd],
                                    lhsT=KT_p[hs, j * P:(j + 1) * P],
                                    rhs=QT_p[hs, 64 * j + pos: 64 * j + pos + wd],
                                    start=(ci == 0), stop=(ci == ncw - 1),
                                    skip_group_check=True)
                        for hh in range(2):
                            nc.scalar.activation(
                                out=pts[hh][:, w * 512:(w + 1) * 512],
                                in_=wins[hh], func=Exp, scale=0.125)
                        for off in win_masks[w]:
                            for hh in range(2):
                                nc.vector.tensor_mul(pts[hh][:, off:off + 64],
                                                     pts[hh][:, off:off + 64],
                                                     mask_sb)

                    # attn @ V: 512-query groups, causal suffix accumulation
                    for hh in range(2):
                        h = pr * 2 + hh
                        pt = pts[hh]
                        for g in range(2):
                            ot = avp.tile([65, 512], F32, tag="av")
                            jmax = 8 * (g + 1)
                            for j in range(jmax):
                                qlo = max(512 * g, 64 * j)
                                wd = 512 * (g + 1) - qlo
                                nc.tensor.matmul(
                                    ot[:, qlo - 512 * g: 512],
                                    lhsT=Vaug_p[:, hh, j, :],
                                    rhs=pt[:, _pt_off(j) + qlo - 64 * j:
                                           _pt_off(j) + qlo - 64 * j + wd],
                                    start=(j == 0), stop=(j == jmax - 1))
                            ob = scr.tile([65, 512], BF16, tag="ob")
                            nc.vector.tensor_copy(ob, ot)
                            tp = trb.tile([P, 4, 96], BF16, tag="tr")
                            for s in range(4):
                                nc.tensor.matmul(tp[:, s, 0:65],
                                                 lhsT=ob[:, s * P:(s + 1) * P],
                                                 rhs=id_bf[0:65, 0:65],
                                                 is_transpose=True,
                                                 start=(s == 0), stop=(s == 3),
                                                 skip_group_check=True)
                            rd = scr.tile([P, 4], F32, tag="rd")
                            nc.vector.reciprocal(rd, tp[:, :, 64])
                            for s in range(4):
                                kb = 4 * g + s
                                nc.vector.scalar_tensor_tensor(
                                    out=xv[:, kb, h * 64:(h + 1) * 64],
                                    in0=tp[:, s, 0:64],
                                    scalar=rd[:, s:s + 1],
                                    in1=xv[:, kb, h * 64:(h + 1) * 64],
                                    op0=MUL, op1=ADD)

            # ---- Phase D: LN2 + MLP + residual over all 1024 tokens ----
            with tc.tile_pool(name="x2tp", bufs=1) as x2tp, \
                 tc.tile_pool(name="h1p", bufs=1) as h1p, \
                 tc.tile_pool(name="w1s", bufs=3) as w1s, \
                 tc.tile_pool(name="w2s", bufs=2) as w2s, \
                 tc.tile_pool(name="lnp2", bufs=2) as lnp2, \
                 tc.tile_pool(name="scr2", bufs=3) as scr2, \
                 tc.tile_pool(name="mmd", bufs=2, space="PSUM") as mmd, \
                 tc.tile_pool(name="trd", bufs=2, space="PSUM") as trd:
                X2T = x2tp.tile([P, NC_DCH, TQ], BF16, tag="x2t")
                for kb in range(NQ):
                    xn2 = lnp2.tile([P, D], BF16, tag="xn2")
                    _layernorm(nc, lnp2, xv[:, kb, :], xn2, eps_sb, g2_t, be2_t)
                    lt = trd.tile([P, NC_DCH, P], BF16, tag="lntr")
                    for c in range(NC_DCH):
                        nc.tensor.matmul(lt[:, c, :], lhsT=xn2[:, c * P:(c + 1) * P],
                                         rhs=id_bf, is_transpose=True,
                                         start=(c == 0), stop=(c == NC_DCH - 1),
                                         skip_group_check=True)
                    nc.vector.tensor_copy(X2T[:, :, kb * P:(kb + 1) * P], lt)
                h1 = h1p.tile([P, 32, TQ], BF16, tag="h1")
                for f in range(32):
                    w1f = w1s.tile([P, NC_DCH, P], BF16, tag="w1f")
                    nc.sync.dma_start(out=w1f, in_=w1t[f])
                    ps = mmd.tile([P, TQ], F32, tag="mm")
                    for g in range(2):
                        for c in range(NC_DCH):
                            nc.tensor.matmul(ps[:, g * 512:(g + 1) * 512],
                                             lhsT=w1f[:, c, :],
                                             rhs=X2T[:, c, g * 512:(g + 1) * 512],
                                             start=(c == 0), stop=(c == NC_DCH - 1))
                    nc.scalar.activation(out=h1[:, f, :], in_=ps, func=Relu,
                                         bias=b1_sb[:, f:f + 1], scale=1.0)
                for dd in range(8):
                    w2d = w2s.tile([P, 32, P], BF16, tag="w2d")
                    nc.sync.dma_start(out=w2d, in_=w2t[dd])
                    ps = mmd.tile([P, TQ], F32, tag="mm")
                    for g in range(2):
                        for fc in range(32):
                            nc.tensor.matmul(ps[:, g * 512:(g + 1) * 512],
                                             lhsT=w2d[:, fc, :],
                                             rhs=h1[:, fc, g * 512:(g + 1) * 512],
                                             start=(fc == 0), stop=(fc == 31))
                    fsb = scr2.tile([P, TQ], BF16, tag="fsb")
                    nc.vector.tensor_scalar_add(fsb, ps, b2_sb[:, dd:dd + 1])
                    ft = trd.tile([P, NQ, P], BF16, tag="fftr")
                    for kb in range(NQ):
                        nc.tensor.matmul(ft[:, kb, :],
                                         lhsT=fsb[:, kb * P:(kb + 1) * P],
                                         rhs=id_bf, is_transpose=True,
                                         start=(kb == 0), stop=(kb == NQ - 1),
                                         skip_group_check=True)
                    nc.vector.tensor_add(xv[:, :, dd * P:(dd + 1) * P],
                                         xv[:, :, dd * P:(dd + 1) * P], ft)
                for kb in range(NQ):
                    nc.sync.dma_start(out=out_d[kb * P:(kb + 1) * P, :], in_=xv[:, kb, :])

    _split_drain_waits(nc)
    return nc


def _split_drain_waits(nc):
    """This walrus build gives every instruction a single hardware wait slot
    (one EVENTS struct per 64B instruction). Tile emits multi-wait
    instructions; move the excess waits onto single-wait NoOps inserted just
    before, on the same engine — identical semantics in program order."""
    for fn in nc.m.functions:
        for blk in fn.blocks:
            insts = blk.instructions
            i = 0
            while i < len(insts):
                inst = insts[i]
                si = inst.sync_info
                if si is not None and len(si.on_wait) > 1:
                    waits = list(si.on_wait)
                    inst.sync_info = mybir.SyncInfo(on_wait=[waits[-1]],
                                                    on_update=list(si.on_update))
                    for w in waits[:-1]:
                        nop = mybir.InstNoOp(name=nc.get_next_instruction_name(),
                                             ins=[], outs=[])
                        nop.engine = inst.engine
                        nop.sync_info = mybir.SyncInfo(on_wait=[w], on_update=[])
                        nc.register_instruction(nop, overwrite=True)
                        insts.insert(i, nop)
                        i += 1
                i += 1


def _prep_inputs(inputs, Wq, Wk, Wv, W1, b1, W2, b2, g1, be1, g2, be2,
                 apply_g1, apply_g2):
    bf = ml_dtypes.bfloat16
    f32 = np.float32
    inputs = np.ascontiguousarray(np.asarray(inputs, f32))
    wq_f = np.asarray(Wq, f32).transpose(1, 0, 2).reshape(D, D)
    wk_f = np.asarray(Wk, f32).transpose(1, 0, 2).reshape(D, D)
    wv_f = np.asarray(Wv, f32).transpose(1, 0, 2).reshape(D, D)

    def pair_tiles(w):  # [D, D] -> [8, 128, 8, 128] (pair, p, chunk, col)
        return np.ascontiguousarray(
            w.reshape(NC_DCH, P, 8, P).transpose(2, 1, 0, 3).astype(bf))

    wq_t, wk_t, wv_t = pair_tiles(wq_f), pair_tiles(wk_f), pair_tiles(wv_f)
    w1_t = np.ascontiguousarray(
        np.asarray(W1, f32).reshape(NC_DCH, P, 32, P).transpose(2, 1, 0, 3).astype(bf))
    w2_t = np.ascontiguousarray(
        np.asarray(W2, f32).reshape(32, P, 8, P).transpose(2, 1, 0, 3).astype(bf))
    b1_t = np.ascontiguousarray(np.asarray(b1, f32).reshape(32, P).T)
    b2_t = np.ascontiguousarray(np.asarray(b2, f32).reshape(8, P).T)

    cc, kk = np.meshgrid(np.arange(P), np.arange(64), indexing="ij")
    in_maps = []
    for c in range(8):
        b, o = divmod(c, 2)
        xb_c = inputs[b]
        xq_c = np.ascontiguousarray(xb_c[o::2, :])
        mask = np.ascontiguousarray((cc <= 2 * kk + o).astype(f32).astype(bf))
        m = {"xb": xb_c, "xq": xq_c, "wq": wq_t, "wk": wk_t, "wv": wv_t,
             "w1t": w1_t, "w2t": w2_t, "b1t": b1_t, "b2t": b2_t, "maskt": mask}
        if apply_g1:
            m["g1v"] = np.asarray(g1, f32)
            m["be1v"] = np.asarray(be1, f32)
        if apply_g2:
            m["g2v"] = np.asarray(g2, f32)
            m["be2v"] = np.asarray(be2, f32)
        in_maps.append(m)
    return in_maps


def _run(inputs, Wq, Wk, Wv, W1, b1, W2, b2, g1, be1, g2, be2, **spmd_kwargs):
    apply_g1 = not (np.all(np.asarray(g1) == 1.0) and np.all(np.asarray(be1) == 0.0))
    apply_g2 = not (np.all(np.asarray(g2) == 1.0) and np.all(np.asarray(be2) == 0.0))
    nc = build_program(apply_g1, apply_g2)
    in_maps = _prep_inputs(inputs, Wq, Wk, Wv, W1, b1, W2, b2, g1, be1, g2, be2,
                           apply_g1, apply_g2)
    res = run_bass_kernel_spmd(nc, in_maps, list(range(8)), **spmd_kwargs)
    out = np.empty((B, T, D), np.float32)
    for c in range(8):
        b, o = divmod(c, 2)
        out[b, o::2, :] = res.results[c]["out"]
    return out, res


def kernel(inputs, Wq, Wk, Wv, W1, b1, W2, b2, g1, be1, g2, be2):
    out, _ = _run(inputs, Wq, Wk, Wv, W1, b1, W2, b2, g1, be1, g2, be2)
    return out


# revision 16
# speedup vs baseline: 1.2953x; 1.0001x over previous
"""GPT block (LN -> causal MHA -> residual -> LN -> MLP -> residual) on 8 trn2 cores.

Sharding: core c = (batch b = c//2, parity o = c%2). Each core owns the
interleaved tokens o::2 of its batch. K/V are computed redundantly by the two
cores of a batch; attention rows and the MLP are token-parallel. No cross-core
communication: outputs are scattered back on the host.

All matmuls run in bf16 (fp32 PSUM accumulation); layernorm/softmax
normalization stats stay fp32. Structured to keep the PE array dense:
- scores for the two heads of a pair are issued interleaved at base
  partitions 0/64 so they run concurrently in disjoint row groups;
- exp runs on the scalar engine from dedicated score PSUM windows so its
  drain never blocks projection matmuls (separate PSUM pools);
- attn@V accumulates 512-query groups with causal suffix matmuls;
- all transposes are bf16 (1 cycle/col); LN normalization is applied by the
  scalar engine (Identity with per-partition scale/bias).
"""

import sys

if "/opt/trn_rl_repo" not in sys.path:
    sys.path.insert(0, "/opt/trn_rl_repo")

import numpy as np
import ml_dtypes

import concourse.bass as bass
import concourse.tile as tile
from concourse import mybir
from concourse.bass_utils import run_bass_kernel_spmd
from concourse.masks import make_identity

B, T, D, H, HD = 4, 2048, 1024, 16, 64
FF = 4 * D
P = 128
NB = T // P        # 16 key blocks
TQ = T // 2        # 1024 query tokens per core
NQ = TQ // P       # 8 query blocks per core
NC_DCH = D // P    # 8 contraction chunks over D
EPS = 1e-5
F32 = mybir.dt.float32
BF16 = mybir.dt.bfloat16
PT_TOTAL = sum(TQ - 64 * j for j in range(NB))  # 8704 = 17 * 512
NWIN = PT_TOTAL // 512  # 17 score windows per head

Exp = mybir.ActivationFunctionType.Exp
Relu = mybir.ActivationFunctionType.Relu
Sqrt = mybir.ActivationFunctionType.Sqrt
Ident = mybir.ActivationFunctionType.Identity
MUL = mybir.AluOpType.mult
ADD = mybir.AluOpType.add


def _pt_off(j):
    return 1024 * j - 32 * j * (j - 1)


def _score_chunks():
    """Flat causal score stream split at key-block and 512-col boundaries.
    Returns [(j, pos, w, off)] with off the packed pt column."""
    chunks = []
    off = 0
    for j in range(NB):
        slen = TQ - 64 * j
        pos = 0
        while pos < slen:
            w = min(512 - (off % 512), slen - pos)
            chunks.append((j, pos, w, off))
            pos += w
            off += w
    assert off == PT_TOTAL
    return chunks


def _layernorm(nc, lnp, src, dst_bf, eps_sb, gtile, btile, on_act=True):
    """LN over rows of src [P, D] f32 -> dst_bf [P, D] bf16. The normalize
    apply runs on the scalar engine (Identity(x*rstd - mu*rstd)) or on the
    vector engine (tensor_scalar) so callers can balance the two."""
    stats = lnp.tile([P, 2, 6], F32, tag="stats")
    for s in range(2):
        nc.vector.bn_stats(out=stats[:, s, :], in_=src[:, s * 512:(s + 1) * 512])
    mv = lnp.tile([P, 2], F32, tag="mv")
    nc.vector.bn_aggr(out=mv, in_=stats)
    rstd = lnp.tile([P, 1], F32, tag="rstd")
    nc.scalar.activation(out=rstd, in_=mv[:, 1:2], func=Sqrt, bias=eps_sb, scale=1.0)
    nc.vector.reciprocal(out=rstd, in_=rstd)
    if on_act:
        nmu = lnp.tile([P, 1], F32, tag="nmu")
        nc.vector.tensor_scalar(out=nmu, in0=mv[:, 0:1], scalar1=rstd, scalar2=-1.0,
                                op0=MUL, op1=MUL)
        nc.scalar.activation(out=dst_bf, in_=src, func=Ident, bias=nmu, scale=rstd)
    else:
        nc.vector.tensor_scalar(out=dst_bf, in0=src, scalar1=mv[:, 0:1],
                                scalar2=rstd, op0=mybir.AluOpType.subtract,
                                op1=MUL)
    if gtile is not None:
        nc.vector.tensor_mul(dst_bf, dst_bf, gtile)
    if btile is not None:
        nc.vector.tensor_add(dst_bf, dst_bf, btile)


def build_program(apply_g1=False, apply_g2=False):
    nc = bass.Bass()
    xb = nc.declare_dram_parameter("xb", [T, D], F32, isOutput=False)
    xq = nc.declare_dram_parameter("xq", [TQ, D], F32, isOutput=False)
    wq = nc.declare_dram_parameter("wq", [8, P, NC_DCH, P], BF16, isOutput=False)
    wk = nc.declare_dram_parameter("wk", [8, P, NC_DCH, P], BF16, isOutput=False)
    wv = nc.declare_dram_parameter("wv", [8, P, NC_DCH, P], BF16, isOutput=False)
    w1t = nc.declare_dram_parameter("w1t", [32, P, NC_DCH, P], BF16, isOutput=False)
    w2t = nc.declare_dram_parameter("w2t", [8, P, 32, P], BF16, isOutput=False)
    b1t = nc.declare_dram_parameter("b1t", [P, 32], F32, isOutput=False)
    b2t = nc.declare_dram_parameter("b2t", [P, 8], F32, isOutput=False)
    maskt = nc.declare_dram_parameter("maskt", [P, 64], BF16, isOutput=False)
    gb = {}
    if apply_g1:
        gb["g1"] = nc.declare_dram_parameter("g1v", [D], F32, isOutput=False)
        gb["be1"] = nc.declare_dram_parameter("be1v", [D], F32, isOutput=False)
    if apply_g2:
        gb["g2"] = nc.declare_dram_parameter("g2v", [D], F32, isOutput=False)
        gb["be2"] = nc.declare_dram_parameter("be2v", [D], F32, isOutput=False)
    out_d = nc.declare_dram_parameter("out", [TQ, D], F32, isOutput=True)

    chunks = _score_chunks()
    win_chunks = [[] for _ in range(NWIN)]
    for c in chunks:
        win_chunks[c[3] // 512].append(c)
    win_masks = [[] for _ in range(NWIN)]
    for j in range(NB):
        off = _pt_off(j)
        win_masks[off // 512].append(off)

    with tile.TileContext(nc) as tc:
        with tc.tile_pool(name="consts", bufs=1) as consts, \
             tc.tile_pool(name="big", bufs=1) as big:
            id_bf = consts.tile([P, P], BF16)
            make_identity(nc, id_bf)
            eps_sb = consts.tile([P, 1], F32)
            nc.vector.memset(eps_sb, EPS)
            mask_sb = consts.tile([P, 64], BF16)
            nc.sync.dma_start(out=mask_sb, in_=maskt[:, :])
            b1_sb = consts.tile([P, 32], F32)
            nc.sync.dma_start(out=b1_sb, in_=b1t[:, :])
            b2_sb = consts.tile([P, 8], F32)
            nc.sync.dma_start(out=b2_sb, in_=b2t[:, :])

            def bcast(name):
                t = consts.tile([P, D], F32, tag=f"bc_{name}")
                src = gb[name]
                ap = bass.AP(tensor=src.tensor if hasattr(src, "tensor") else src[:].tensor,
                             offset=src[:].offset, ap=[[0, P]] + list(src[:].ap))
                nc.sync.dma_start(out=t, in_=ap)
                return t

            g1_t = bcast("g1") if apply_g1 else None
            be1_t = bcast("be1") if apply_g1 else None
            g2_t = bcast("g2") if apply_g2 else None
            be2_t = bcast("be2") if apply_g2 else None

            XT = big.tile([P, NC_DCH, T], BF16)    # LN1(xb)^T
            XQT = big.tile([P, NC_DCH, TQ], BF16)  # strided query columns of XT
            xv = big.tile([P, NQ, D], F32)         # residual stream, my tokens

            # ---- Phase A: layernorm1 + transposes ----
            with tc.tile_pool(name="lnp", bufs=4) as lnp, \
                 tc.tile_pool(name="lnsrc", bufs=4) as lnsrc, \
                 tc.tile_pool(name="trp", bufs=1, space="PSUM") as trp:
                for blk in range(NB):
                    x_t = lnsrc.tile([P, D], F32, tag="xsrc")
                    nc.sync.dma_start(out=x_t, in_=xb[blk * P:(blk + 1) * P, :])
                    xn = lnp.tile([P, D], BF16, tag="xn")
                    _layernorm(nc, lnp, x_t, xn, eps_sb, g1_t, be1_t,
                               on_act=(blk % 2 == 0))
                    lt = trp.tile([P, NC_DCH, P], BF16, tag="lntr")
                    for c in range(NC_DCH):
                        nc.tensor.matmul(lt[:, c, :], lhsT=xn[:, c * P:(c + 1) * P],
                                         rhs=id_bf, is_transpose=True,
                                         start=(c == 0), stop=(c == NC_DCH - 1),
                                         skip_group_check=True)
                    nc.vector.tensor_copy(XT[:, :, blk * P:(blk + 1) * P], lt)
                for kb in range(NQ):
                    nc.sync.dma_start(out=xv[:, kb, :], in_=xq[kb * P:(kb + 1) * P, :])
                    xnq = lnp.tile([P, D], BF16, tag="xn")
                    _layernorm(nc, lnp, xv[:, kb, :], xnq, eps_sb, g1_t, be1_t,
                               on_act=(kb % 2 == 0))
                    ltq = trp.tile([P, NC_DCH, P], BF16, tag="lntr")
                    for c in range(NC_DCH):
                        nc.tensor.matmul(ltq[:, c, :], lhsT=xnq[:, c * P:(c + 1) * P],
                                         rhs=id_bf, is_transpose=True,
                                         start=(c == 0), stop=(c == NC_DCH - 1),
                                         skip_group_check=True)
                    nc.vector.tensor_copy(XQT[:, :, kb * P:(kb + 1) * P], ltq)

            # ---- Phase B/C: per head-pair projections + attention ----
            with tc.tile_pool(name="wp", bufs=2) as wp, \
                 tc.tile_pool(name="ap", bufs=2) as apool, \
                 tc.tile_pool(name="ptp", bufs=1) as ptp, \
                 tc.tile_pool(name="scr", bufs=4) as scr, \
                 tc.tile_pool(name="pp", bufs=2, space="PSUM") as pp, \
                 tc.tile_pool(name="sw", bufs=2, space="PSUM") as sw, \
                 tc.tile_pool(name="avp", bufs=1, space="PSUM") as avp, \
                 tc.tile_pool(name="trb", bufs=2, space="PSUM") as trb:
                for pr in range(8):
                    wq_p = wp.tile([P, NC_DCH, P], BF16, tag="wq")
                    nc.sync.dma_start(out=wq_p, in_=wq[pr])
                    wk_p = wp.tile([P, NC_DCH, P], BF16, tag="wk")
                    nc.sync.dma_start(out=wk_p, in_=wk[pr])
                    wv_p = wp.tile([P, NC_DCH, P], BF16, tag="wv")
                    nc.sync.dma_start(out=wv_p, in_=wv[pr])

                    KT_p = apool.tile([P, T], BF16, tag="kt")
                    for tg in range(4):
                        ps = pp.tile([P, 512], F32, tag="mm")
                        for c in range(NC_DCH):
                            nc.tensor.matmul(ps, lhsT=wk_p[:, c, :],
                                             rhs=XT[:, c, tg * 512:(tg + 1) * 512],
                                             start=(c == 0), stop=(c == NC_DCH - 1))
                        nc.vector.tensor_copy(KT_p[:, tg * 512:(tg + 1) * 512], ps)

                    Vaug_p = apool.tile([P, 2, NB, 65], BF16, tag="vaug")
                    nc.vector.memset(Vaug_p[:, :, :, 64:65], 1.0)
                    for tg in range(4):
                        ps = pp.tile([P, 512], F32, tag="mm")
                        for c in range(NC_DCH):
                            nc.tensor.matmul(ps, lhsT=wv_p[:, c, :],
                                             rhs=XT[:, c, tg * 512:(tg + 1) * 512],
                                             start=(c == 0), stop=(c == NC_DCH - 1))
                        vt_sb = scr.tile([P, 512], BF16, tag="vt")
                        nc.vector.tensor_copy(vt_sb, ps)
                        for hh in range(2):
                            # one accumulation group per bank, uniform base
                            # partition within the group (mixed bases wedge
                            # the PE)
                            tb = trb.tile([P, 4, 64], BF16, tag="tr")
                            for s in range(4):
                                nc.tensor.matmul(
                                    tb[:, s, :],
                                    lhsT=vt_sb[hh * 64:(hh + 1) * 64, s * P:(s + 1) * P],
                                    rhs=id_bf[hh * 64:(hh + 1) * 64, hh * 64:hh * 64 + 64],
                                    is_transpose=True,
                                    start=(s == 0), stop=(s == 3),
                                    skip_group_check=True)
                            nc.vector.tensor_copy(
                                Vaug_p[:, hh, tg * 4:(tg + 1) * 4, 0:64], tb)

                    QT_p = apool.tile([P, TQ], BF16, tag="qt")
                    for tg in range(2):
                        ps = pp.tile([P, 512], F32, tag="mm")
                        for c in range(NC_DCH):
                            nc.tensor.matmul(ps, lhsT=wq_p[:, c, :],
                                             rhs=XQT[:, c, tg * 512:(tg + 1) * 512],
                                             start=(c == 0), stop=(c == NC_DCH - 1))
                        nc.vector.tensor_copy(QT_p[:, tg * 512:(tg + 1) * 512], ps)

                    # scores: both heads interleaved (row groups 0-63 / 64-127)
                    pt0 = ptp.tile([P, PT_TOTAL], BF16, tag="pt0")
                    pt1 = ptp.tile([P, PT_TOTAL], BF16, tag="pt1")
                    pts = [pt0, pt1]
                    for w in range(NWIN):
                        win0 = sw.tile([P, 512], F32, tag="sw")
                        win1 = sw.tile([P, 512], F32, tag="sw")
                        wins = [win0, win1]
                        ncw = len(win_chunks[w])
                        for ci, (j, pos, wd, off) in enumerate(win_chunks[w]):
                            wcol = off % 512
                            for hh in range(2):
                                hs = slice(hh * 64, (hh + 1) * 64)
                                nc.tensor.matmul(
                                    wins[hh][:, wcol:wcol + wd],
                                    lhsT=KT_p[hs, j * P:(j + 1) * P],
                                    rhs=QT_p[hs, 64 * j + pos: 64 * j + pos + wd],
                                    start=(ci == 0), stop=(ci == ncw - 1),
                                    skip_group_check=True)
                        for hh in range(2):
                            nc.scalar.activation(
                                out=pts[hh][:, w * 512:(w + 1) * 512],
                                in_=wins[hh], func=Exp, scale=0.125)
                        for off in win_masks[w]:
                            for hh in range(2):
                                nc.vector.tensor_mul(pts[hh][:, off:off + 64],
                                                     pts[hh][:, off:off + 64],
                                                     mask_sb)

                    # attn @ V: 512-query groups, causal suffix accumulation
                    for hh in range(2):
                        h = pr * 2 + hh
                        pt = pts[hh]
                        for g in range(2):
                            ot = avp.tile([65, 512], F32, tag="av")
                            jmax = 8 * (g + 1)
                            for j in range(jmax):
                                qlo = max(512 * g, 64 * j)
                                wd = 512 * (g + 1) - qlo
                                nc.tensor.matmul(
                                    ot[:, qlo - 512 * g: 512],
                                    lhsT=Vaug_p[:, hh, j, :],
                                    rhs=pt[:, _pt_off(j) + qlo - 64 * j:
                                           _pt_off(j) + qlo - 64 * j + wd],
                                    start=(j == 0), stop=(j == jmax - 1))
                            ob = scr.tile([65, 512], BF16, tag="ob")
                            nc.vector.tensor_copy(ob, ot)
                            tp = trb.tile([P, 4, 96], BF16, tag="tr")
                            for s in range(4):
                                nc.tensor.matmul(tp[:, s, 0:65],
                                                 lhsT=ob[:, s * P:(s + 1) * P],
                                                 rhs=id_bf[0:65, 0:65],
                                                 is_transpose=True,
                                                 start=(s == 0), stop=(s == 3),
                                                 skip_group_check=True)
                            rd = scr.tile([P, 4], F32, tag="rd")
                            nc.vector.reciprocal(rd, tp[:, :, 64])
                            for s in range(4):
                                kb = 4 * g + s
                                nc.vector.scalar_tensor_tensor(
                                    out=xv[:, kb, h * 64:(h + 1) * 64],
                                    in0=tp[:, s, 0:64],
                                    scalar=rd[:, s:s + 1],
                                    in1=xv[:, kb, h * 64:(h + 1) * 64],
                                    op0=MUL, op1=ADD)

            # ---- Phase D: LN2 + MLP + residual over all 1024 tokens ----
            with tc.tile_pool(name="x2tp", bufs=1) as x2tp, \
                 tc.tile_pool(name="h1p", bufs=1) as h1p, \
                 tc.tile_pool(name="w1s", bufs=3) as w1s, \
                 tc.tile_pool(name="w2s", bufs=2) as w2s, \
                 tc.tile_pool(name="lnp2", bufs=2) as lnp2, \
                 tc.tile_pool(name="scr2", bufs=3) as scr2, \
                 tc.tile_pool(name="mmd", bufs=2, space="PSUM") as mmd, \
                 tc.tile_pool(name="trd", bufs=2, space="PSUM") as trd:
                X2T = x2tp.tile([P, NC_DCH, TQ], BF16, tag="x2t")
                for kb in range(NQ):
                    xn2 = lnp2.tile([P, D], BF16, tag="xn2")
                    _layernorm(nc, lnp2, xv[:, kb, :], xn2, eps_sb, g2_t, be2_t)
                    lt = trd.tile([P, NC_DCH, P], BF16, tag="lntr")
                    for c in range(NC_DCH):
                        nc.tensor.matmul(lt[:, c, :], lhsT=xn2[:, c * P:(c + 1) * P],
                                         rhs=id_bf, is_transpose=True,
                                         start=(c == 0), stop=(c == NC_DCH - 1),
                                         skip_group_check=True)
                    nc.vector.tensor_copy(X2T[:, :, kb * P:(kb + 1) * P], lt)
                h1 = h1p.tile([P, 32, TQ], BF16, tag="h1")
                for f in range(32):
                    w1f = w1s.tile([P, NC_DCH, P], BF16, tag="w1f")
                    nc.sync.dma_start(out=w1f, in_=w1t[f])
                    ps = mmd.tile([P, TQ], F32, tag="mm")
                    for g in range(2):
                        for c in range(NC_DCH):
                            nc.tensor.matmul(ps[:, g * 512:(g + 1) * 512],
                                             lhsT=w1f[:, c, :],
                                             rhs=X2T[:, c, g * 512:(g + 1) * 512],
                                             start=(c == 0), stop=(c == NC_DCH - 1))
                    nc.scalar.activation(out=h1[:, f, :], in_=ps, func=Relu,
                                         bias=b1_sb[:, f:f + 1], scale=1.0)
                for dd in range(8):
                    w2d = w2s.tile([P, 32, P], BF16, tag="w2d")
                    nc.sync.dma_start(out=w2d, in_=w2t[dd])
                    ps = mmd.tile([P, TQ], F32, tag="mm")
                    for g in range(2):
                        for fc in range(32):
                            nc.tensor.matmul(ps[:, g * 512:(g + 1) * 512],
                                             lhsT=w2d[:, fc, :],
                                             rhs=h1[:, fc, g * 512:(g + 1) * 512],
                                             start=(fc == 0), stop=(fc == 31))
                    fsb = scr2.tile([P, TQ], BF16, tag="fsb")
                    nc.vector.tensor_scalar_add(fsb, ps, b2_sb[:, dd:dd + 1])
                    ft = trd.tile([P, NQ, P], BF16, tag="fftr")
                    for kb in range(NQ):
                        nc.tensor.matmul(ft[:, kb, :],
                                         lhsT=fsb[:, kb * P:(kb + 1) * P],
                                         rhs=id_bf, is_transpose=True,
                                         start=(kb == 0), stop=(kb == NQ - 1),
                                         skip_group_check=True)
                    nc.vector.tensor_add(xv[:, :, dd * P:(dd + 1) * P],
                                         xv[:, :, dd * P:(dd + 1) * P], ft)
                for kb in range(NQ):
                    nc.sync.dma_start(out=out_d[kb * P:(kb + 1) * P, :], in_=xv[:, kb, :])

    _split_drain_waits(nc)
    return nc


def _split_drain_waits(nc):
    """This walrus build gives every instruction a single hardware wait slot
    (one EVENTS struct per 64B instruction). Tile emits multi-wait
    instructions; move the excess waits onto single-wait NoOps inserted just
    before, on the same engine — identical semantics in program order."""
    for fn in nc.m.functions:
        for blk in fn.blocks:
            insts = blk.instructions
            i = 0
            while i < len(insts):
                inst = insts[i]
                si = inst.sync_info
                if si is not None and len(si.on_wait) > 1:
                    waits = list(si.on_wait)
                    inst.sync_info = mybir.SyncInfo(on_wait=[waits[-1]],
                                                    on_update=list(si.on_update))
                    for w in waits[:-1]:
                        nop = mybir.InstNoOp(name=nc.get_next_instruction_name(),
                                             ins=[], outs=[])
                        nop.engine = inst.engine
                        nop.sync_info = mybir.SyncInfo(on_wait=[w], on_update=[])
                        nc.register_instruction(nop, overwrite=True)
                        insts.insert(i, nop)
                        i += 1
                i += 1


def _prep_inputs(inputs, Wq, Wk, Wv, W1, b1, W2, b2, g1, be1, g2, be2,
                 apply_g1, apply_g2):
    bf = ml_dtypes.bfloat16
    f32 = np.float32
    inputs = np.ascontiguousarray(np.asarray(inputs, f32))
    wq_f = np.asarray(Wq, f32).transpose(1, 0, 2).reshape(D, D)
    wk_f = np.asarray(Wk, f32).transpose(1, 0, 2).reshape(D, D)
    wv_f = np.asarray(Wv, f32).transpose(1, 0, 2).reshape(D, D)

    def pair_tiles(w):  # [D, D] -> [8, 128, 8, 128] (pair, p, chunk, col)
        return np.ascontiguousarray(
            w.reshape(NC_DCH, P, 8, P).transpose(2, 1, 0, 3).astype(bf))

    wq_t, wk_t, wv_t = pair_tiles(wq_f), pair_tiles(wk_f), pair_tiles(wv_f)
    w1_t = np.ascontiguousarray(
        np.asarray(W1, f32).reshape(NC_DCH, P, 32, P).transpose(2, 1, 0, 3).astype(bf))
    w2_t = np.ascontiguousarray(
        np.asarray(W2, f32).reshape(32, P, 8, P).transpose(2, 1, 0, 3).astype(bf))
    b1_t = np.ascontiguousarray(np.asarray(b1, f32).reshape(32, P).T)
    b2_t = np.ascontiguousarray(np.asarray(b2, f32).reshape(8, P).T)

    cc, kk = np.meshgrid(np.arange(P), np.arange(64), indexing="ij")
    in_maps = []
    for c in range(8):
        b, o = divmod(c, 2)
        xb_c = inputs[b]
        xq_c = np.ascontiguousarray(xb_c[o::2, :])
        mask = np.ascontiguousarray((cc <= 2 * kk + o).astype(f32).astype(bf))
        m = {"xb": xb_c, "xq": xq_c, "wq": wq_t, "wk": wk_t, "wv": wv_t,
             "w1t": w1_t, "w2t": w2_t, "b1t": b1_t, "b2t": b2_t, "maskt": mask}
        if apply_g1:
            m["g1v"] = np.asarray(g1, f32)
            m["be1v"] = np.asarray(be1, f32)
        if apply_g2:
            m["g2v"] = np.asarray(g2, f32)
            m["be2v"] = np.asarray(be2, f32)
        in_maps.append(m)
    return in_maps


def _run(inputs, Wq, Wk, Wv, W1, b1, W2, b2, g1, be1, g2, be2, **spmd_kwargs):
    apply_g1 = not (np.all(np.asarray(g1) == 1.0) and np.all(np.asarray(be1) == 0.0))
    apply_g2 = not (np.all(np.asarray(g2) == 1.0) and np.all(np.asarray(be2) == 0.0))
    nc = build_program(apply_g1, apply_g2)
    in_maps = _prep_inputs(inputs, Wq, Wk, Wv, W1, b1, W2, b2, g1, be1, g2, be2,
                           apply_g1, apply_g2)
    res = run_bass_kernel_spmd(nc, in_maps, list(range(8)), **spmd_kwargs)
    out = np.empty((B, T, D), np.float32)
    for c in range(8):
        b, o = divmod(c, 2)
        out[b, o::2, :] = res.results[c]["out"]
    return out, res


def kernel(inputs, Wq, Wk, Wv, W1, b1, W2, b2, g1, be1, g2, be2):
    out, _ = _run(inputs, Wq, Wk, Wv, W1, b1, W2, b2, g1, be1, g2, be2)
    return out


# revision 17
# speedup vs baseline: 1.2985x; 1.0024x over previous
"""GPT block (LN -> causal MHA -> residual -> LN -> MLP -> residual) on 8 trn2 cores.

Sharding: core c = (batch b = c//2, parity o = c%2). Each core owns the
interleaved tokens o::2 of its batch. K/V are computed redundantly by the two
cores of a batch; attention rows and the MLP are token-parallel. No cross-core
communication: outputs are scattered back on the host.

All matmuls run in bf16 (fp32 PSUM accumulation); layernorm/softmax
normalization stats stay fp32. Structured to keep the PE array dense:
- scores for the two heads of a pair are issued interleaved at base
  partitions 0/64 so they run concurrently in disjoint row groups;
- exp runs on the scalar engine from dedicated score PSUM windows so its
  drain never blocks projection matmuls (separate PSUM pools);
- attn@V accumulates 512-query groups with causal suffix matmuls;
- all transposes are bf16 (1 cycle/col); LN normalization is applied by the
  scalar engine (Identity with per-partition scale/bias).
"""

import sys

if "/opt/trn_rl_repo" not in sys.path:
    sys.path.insert(0, "/opt/trn_rl_repo")

import numpy as np
import ml_dtypes

import concourse.bass as bass
import concourse.tile as tile
from concourse import mybir
from concourse.bass_utils import run_bass_kernel_spmd
from concourse.masks import make_identity

B, T, D, H, HD = 4, 2048, 1024, 16, 64
FF = 4 * D
P = 128
NB = T // P        # 16 key blocks
TQ = T // 2        # 1024 query tokens per core
NQ = TQ // P       # 8 query blocks per core
NC_DCH = D // P    # 8 contraction chunks over D
EPS = 1e-5
F32 = mybir.dt.float32
BF16 = mybir.dt.bfloat16
PT_TOTAL = sum(TQ - 64 * j for j in range(NB))  # 8704 = 17 * 512
NWIN = PT_TOTAL // 512  # 17 score windows per head

Exp = mybir.ActivationFunctionType.Exp
Relu = mybir.ActivationFunctionType.Relu
Sqrt = mybir.ActivationFunctionType.Sqrt
Ident = mybir.ActivationFunctionType.Identity
MUL = mybir.AluOpType.mult
ADD = mybir.AluOpType.add


def _pt_off(j):
    return 1024 * j - 32 * j * (j - 1)


def _score_chunks():
    """Flat causal score stream split at key-block and 512-col boundaries.
    Returns [(j, pos, w, off)] with off the packed pt column."""
    chunks = []
    off = 0
    for j in range(NB):
        slen = TQ - 64 * j
        pos = 0
        while pos < slen:
            w = min(512 - (off % 512), slen - pos)
            chunks.append((j, pos, w, off))
            pos += w
            off += w
    assert off == PT_TOTAL
    return chunks


def _layernorm(nc, lnp, src, dst_bf, eps_sb, gtile, btile, on_act=True):
    """LN over rows of src [P, D] f32 -> dst_bf [P, D] bf16. The normalize
    apply runs on the scalar engine (Identity(x*rstd - mu*rstd)) or on the
    vector engine (tensor_scalar) so callers can balance the two."""
    stats = lnp.tile([P, 2, 6], F32, tag="stats")
    for s in range(2):
        nc.vector.bn_stats(out=stats[:, s, :], in_=src[:, s * 512:(s + 1) * 512])
    mv = lnp.tile([P, 2], F32, tag="mv")
    nc.vector.bn_aggr(out=mv, in_=stats)
    rstd = lnp.tile([P, 1], F32, tag="rstd")
    nc.scalar.activation(out=rstd, in_=mv[:, 1:2], func=Sqrt, bias=eps_sb, scale=1.0)
    nc.vector.reciprocal(out=rstd, in_=rstd)
    if on_act:
        nmu = lnp.tile([P, 1], F32, tag="nmu")
        nc.vector.tensor_scalar(out=nmu, in0=mv[:, 0:1], scalar1=rstd, scalar2=-1.0,
                                op0=MUL, op1=MUL)
        nc.scalar.activation(out=dst_bf, in_=src, func=Ident, bias=nmu, scale=rstd)
    else:
        nc.vector.tensor_scalar(out=dst_bf, in0=src, scalar1=mv[:, 0:1],
                                scalar2=rstd, op0=mybir.AluOpType.subtract,
                                op1=MUL)
    if gtile is not None:
        nc.vector.tensor_mul(dst_bf, dst_bf, gtile)
    if btile is not None:
        nc.vector.tensor_add(dst_bf, dst_bf, btile)


def build_program(apply_g1=False, apply_g2=False):
    nc = bass.Bass()
    xb = nc.declare_dram_parameter("xb", [T, D], F32, isOutput=False)
    xq = nc.declare_dram_parameter("xq", [TQ, D], F32, isOutput=False)
    wq = nc.declare_dram_parameter("wq", [8, P, NC_DCH, P], BF16, isOutput=False)
    wk = nc.declare_dram_parameter("wk", [8, P, NC_DCH, P], BF16, isOutput=False)
    wv = nc.declare_dram_parameter("wv", [8, P, NC_DCH, P], BF16, isOutput=False)
    w1t = nc.declare_dram_parameter("w1t", [32, P, NC_DCH, P], BF16, isOutput=False)
    w2t = nc.declare_dram_parameter("w2t", [8, P, 32, P], BF16, isOutput=False)
    b1t = nc.declare_dram_parameter("b1t", [P, 32], F32, isOutput=False)
    b2t = nc.declare_dram_parameter("b2t", [P, 8], F32, isOutput=False)
    maskt = nc.declare_dram_parameter("maskt", [P, 64], BF16, isOutput=False)
    gb = {}
    if apply_g1:
        gb["g1"] = nc.declare_dram_parameter("g1v", [D], F32, isOutput=False)
        gb["be1"] = nc.declare_dram_parameter("be1v", [D], F32, isOutput=False)
    if apply_g2:
        gb["g2"] = nc.declare_dram_parameter("g2v", [D], F32, isOutput=False)
        gb["be2"] = nc.declare_dram_parameter("be2v", [D], F32, isOutput=False)
    out_d = nc.declare_dram_parameter("out", [TQ, D], F32, isOutput=True)

    chunks = _score_chunks()
    win_chunks = [[] for _ in range(NWIN)]
    for c in chunks:
        win_chunks[c[3] // 512].append(c)
    win_masks = [[] for _ in range(NWIN)]
    for j in range(NB):
        off = _pt_off(j)
        win_masks[off // 512].append(off)

    with tile.TileContext(nc) as tc:
        with tc.tile_pool(name="consts", bufs=1) as consts, \
             tc.tile_pool(name="big", bufs=1) as big, \
             tc.tile_pool(name="trx", bufs=2, space="PSUM") as trx:
            id_bf = consts.tile([P, P], BF16)
            make_identity(nc, id_bf)
            eps_sb = consts.tile([P, 1], F32)
            nc.vector.memset(eps_sb, EPS)
            mask_sb = consts.tile([P, 64], BF16)
            nc.sync.dma_start(out=mask_sb, in_=maskt[:, :])
            b1_sb = consts.tile([P, 32], F32)
            nc.sync.dma_start(out=b1_sb, in_=b1t[:, :])
            b2_sb = consts.tile([P, 8], F32)
            nc.sync.dma_start(out=b2_sb, in_=b2t[:, :])

            def bcast(name):
                t = consts.tile([P, D], F32, tag=f"bc_{name}")
                src = gb[name]
                ap = bass.AP(tensor=src.tensor if hasattr(src, "tensor") else src[:].tensor,
                             offset=src[:].offset, ap=[[0, P]] + list(src[:].ap))
                nc.sync.dma_start(out=t, in_=ap)
                return t

            g1_t = bcast("g1") if apply_g1 else None
            be1_t = bcast("be1") if apply_g1 else None
            g2_t = bcast("g2") if apply_g2 else None
            be2_t = bcast("be2") if apply_g2 else None

            XT = big.tile([P, NC_DCH, T], BF16)    # LN1(xb)^T
            XQT = big.tile([P, NC_DCH, TQ], BF16)  # strided query columns of XT
            xv = big.tile([P, NQ, D], F32)         # residual stream, my tokens

            # ---- Phase A: layernorm1 + transposes ----
            with tc.tile_pool(name="lnp", bufs=4) as lnp, \
                 tc.tile_pool(name="lnsrc", bufs=4) as lnsrc:
                for blk in range(NB):
                    x_t = lnsrc.tile([P, D], F32, tag="xsrc")
                    nc.sync.dma_start(out=x_t, in_=xb[blk * P:(blk + 1) * P, :])
                    xn = lnp.tile([P, D], BF16, tag="xn")
                    _layernorm(nc, lnp, x_t, xn, eps_sb, g1_t, be1_t,
                               on_act=(blk % 2 == 0))
                    lt = trx.tile([P, NC_DCH, P], BF16, tag="tr")
                    for c in range(NC_DCH):
                        nc.tensor.matmul(lt[:, c, :], lhsT=xn[:, c * P:(c + 1) * P],
                                         rhs=id_bf, is_transpose=True,
                                         start=(c == 0), stop=(c == NC_DCH - 1),
                                         skip_group_check=True)
                    nc.vector.tensor_copy(XT[:, :, blk * P:(blk + 1) * P], lt)
                for kb in range(NQ):
                    nc.sync.dma_start(out=xv[:, kb, :], in_=xq[kb * P:(kb + 1) * P, :])
                    xnq = lnp.tile([P, D], BF16, tag="xn")
                    _layernorm(nc, lnp, xv[:, kb, :], xnq, eps_sb, g1_t, be1_t,
                               on_act=(kb % 2 == 0))
                    ltq = trx.tile([P, NC_DCH, P], BF16, tag="tr")
                    for c in range(NC_DCH):
                        nc.tensor.matmul(ltq[:, c, :], lhsT=xnq[:, c * P:(c + 1) * P],
                                         rhs=id_bf, is_transpose=True,
                                         start=(c == 0), stop=(c == NC_DCH - 1),
                                         skip_group_check=True)
                    nc.vector.tensor_copy(XQT[:, :, kb * P:(kb + 1) * P], ltq)

            # ---- Phase B/C: per head-pair projections + attention ----
            with tc.tile_pool(name="wp", bufs=2) as wp, \
                 tc.tile_pool(name="ap", bufs=2) as apool, \
                 tc.tile_pool(name="ptp", bufs=1) as ptp, \
                 tc.tile_pool(name="scr", bufs=4) as scr, \
                 tc.tile_pool(name="pp", bufs=2, space="PSUM") as pp, \
                 tc.tile_pool(name="sw", bufs=2, space="PSUM") as sw, \
                 tc.tile_pool(name="avp", bufs=2, space="PSUM") as avp:
                for pr in range(8):
                    wq_p = wp.tile([P, NC_DCH, P], BF16, tag="wq")
                    nc.sync.dma_start(out=wq_p, in_=wq[pr])
                    wk_p = wp.tile([P, NC_DCH, P], BF16, tag="wk")
                    nc.sync.dma_start(out=wk_p, in_=wk[pr])
                    wv_p = wp.tile([P, NC_DCH, P], BF16, tag="wv")
                    nc.sync.dma_start(out=wv_p, in_=wv[pr])

                    KT_p = apool.tile([P, T], BF16, tag="kt")
                    for tg in range(4):
                        ps = pp.tile([P, 512], F32, tag="mm")
                        for c in range(NC_DCH):
                            nc.tensor.matmul(ps, lhsT=wk_p[:, c, :],
                                             rhs=XT[:, c, tg * 512:(tg + 1) * 512],
                                             start=(c == 0), stop=(c == NC_DCH - 1))
                        nc.vector.tensor_copy(KT_p[:, tg * 512:(tg + 1) * 512], ps)

                    Vaug_p = apool.tile([P, 2, NB, 65], BF16, tag="vaug")
                    nc.vector.memset(Vaug_p[:, :, :, 64:65], 1.0)
                    for tg in range(4):
                        ps = pp.tile([P, 512], F32, tag="mm")
                        for c in range(NC_DCH):
                            nc.tensor.matmul(ps, lhsT=wv_p[:, c, :],
                                             rhs=XT[:, c, tg * 512:(tg + 1) * 512],
                                             start=(c == 0), stop=(c == NC_DCH - 1))
                        vt_sb = scr.tile([P, 512], BF16, tag="vt")
                        nc.vector.tensor_copy(vt_sb, ps)
                        for hh in range(2):
                            # one accumulation group per bank, uniform base
                            # partition within the group (mixed bases wedge
                            # the PE)
                            tb = trx.tile([P, 4, 64], BF16, tag="tr")
                            for s in range(4):
                                nc.tensor.matmul(
                                    tb[:, s, :],
                                    lhsT=vt_sb[hh * 64:(hh + 1) * 64, s * P:(s + 1) * P],
                                    rhs=id_bf[hh * 64:(hh + 1) * 64, hh * 64:hh * 64 + 64],
                                    is_transpose=True,
                                    start=(s == 0), stop=(s == 3),
                                    skip_group_check=True)
                            nc.vector.tensor_copy(
                                Vaug_p[:, hh, tg * 4:(tg + 1) * 4, 0:64], tb)

                    QT_p = apool.tile([P, TQ], BF16, tag="qt")
                    for tg in range(2):
                        ps = pp.tile([P, 512], F32, tag="mm")
                        for c in range(NC_DCH):
                            nc.tensor.matmul(ps, lhsT=wq_p[:, c, :],
                                             rhs=XQT[:, c, tg * 512:(tg + 1) * 512],
                                             start=(c == 0), stop=(c == NC_DCH - 1))
                        nc.vector.tensor_copy(QT_p[:, tg * 512:(tg + 1) * 512], ps)

                    # scores: both heads interleaved (row groups 0-63 / 64-127)
                    pt0 = ptp.tile([P, PT_TOTAL], BF16, tag="pt0")
                    pt1 = ptp.tile([P, PT_TOTAL], BF16, tag="pt1")
                    pts = [pt0, pt1]
                    for w in range(NWIN):
                        win0 = sw.tile([P, 512], F32, tag="sw")
                        win1 = sw.tile([P, 512], F32, tag="sw")
                        wins = [win0, win1]
                        ncw = len(win_chunks[w])
                        for ci, (j, pos, wd, off) in enumerate(win_chunks[w]):
                            wcol = off % 512
                            for hh in range(2):
                                hs = slice(hh * 64, (hh + 1) * 64)
                                nc.tensor.matmul(
                                    wins[hh][:, wcol:wcol + wd],
                                    lhsT=KT_p[hs, j * P:(j + 1) * P],
                                    rhs=QT_p[hs, 64 * j + pos: 64 * j + pos + wd],
                                    start=(ci == 0), stop=(ci == ncw - 1),
                                    skip_group_check=True)
                        for hh in range(2):
                            nc.scalar.activation(
                                out=pts[hh][:, w * 512:(w + 1) * 512],
                                in_=wins[hh], func=Exp, scale=0.125)
                        for off in win_masks[w]:
                            for hh in range(2):
                                nc.vector.tensor_mul(pts[hh][:, off:off + 64],
                                                     pts[hh][:, off:off + 64],
                                                     mask_sb)

                    # attn @ V: 512-query groups, causal suffix accumulation
                    for hh in range(2):
                        h = pr * 2 + hh
                        pt = pts[hh]
                        for g in range(2):
                            ot = avp.tile([65, 512], F32, tag="av")
                            jmax = 8 * (g + 1)
                            for j in range(jmax):
                                qlo = max(512 * g, 64 * j)
                                wd = 512 * (g + 1) - qlo
                                nc.tensor.matmul(
                                    ot[:, qlo - 512 * g: 512],
                                    lhsT=Vaug_p[:, hh, j, :],
                                    rhs=pt[:, _pt_off(j) + qlo - 64 * j:
                                           _pt_off(j) + qlo - 64 * j + wd],
                                    start=(j == 0), stop=(j == jmax - 1))
                            ob = scr.tile([65, 512], BF16, tag="ob")
                            nc.vector.tensor_copy(ob, ot)
                            tp = trx.tile([P, 4, 96], BF16, tag="tr")
                            for s in range(4):
                                nc.tensor.matmul(tp[:, s, 0:65],
                                                 lhsT=ob[:, s * P:(s + 1) * P],
                                                 rhs=id_bf[0:65, 0:65],
                                                 is_transpose=True,
                                                 start=(s == 0), stop=(s == 3),
                                                 skip_group_check=True)
                            rd = scr.tile([P, 4], F32, tag="rd")
                            nc.vector.reciprocal(rd, tp[:, :, 64])
                            for s in range(4):
                                kb = 4 * g + s
                                nc.vector.scalar_tensor_tensor(
                                    out=xv[:, kb, h * 64:(h + 1) * 64],
                                    in0=tp[:, s, 0:64],
                                    scalar=rd[:, s:s + 1],
                                    in1=xv[:, kb, h * 64:(h + 1) * 64],
                                    op0=MUL, op1=ADD)

            # ---- Phase D: LN2 + MLP + residual over all 1024 tokens ----
            with tc.tile_pool(name="x2tp", bufs=1) as x2tp, \
                 tc.tile_pool(name="h1p", bufs=1) as h1p, \
                 tc.tile_pool(name="w1s", bufs=3) as w1s, \
                 tc.tile_pool(name="w2s", bufs=2) as w2s, \
                 tc.tile_pool(name="lnp2", bufs=2) as lnp2, \
                 tc.tile_pool(name="scr2", bufs=3) as scr2, \
                 tc.tile_pool(name="mmd", bufs=3, space="PSUM") as mmd:
                X2T = x2tp.tile([P, NC_DCH, TQ], BF16, tag="x2t")
                for kb in range(NQ):
                    xn2 = lnp2.tile([P, D], BF16, tag="xn2")
                    _layernorm(nc, lnp2, xv[:, kb, :], xn2, eps_sb, g2_t, be2_t)
                    lt = trx.tile([P, NC_DCH, P], BF16, tag="tr")
                    for c in range(NC_DCH):
                        nc.tensor.matmul(lt[:, c, :], lhsT=xn2[:, c * P:(c + 1) * P],
                                         rhs=id_bf, is_transpose=True,
                                         start=(c == 0), stop=(c == NC_DCH - 1),
                                         skip_group_check=True)
                    nc.vector.tensor_copy(X2T[:, :, kb * P:(kb + 1) * P], lt)
                h1 = h1p.tile([P, 32, TQ], BF16, tag="h1")
                for f in range(32):
                    w1f = w1s.tile([P, NC_DCH, P], BF16, tag="w1f")
                    nc.sync.dma_start(out=w1f, in_=w1t[f])
                    ps = mmd.tile([P, TQ], F32, tag="mm")
                    for g in range(2):
                        for c in range(NC_DCH):
                            nc.tensor.matmul(ps[:, g * 512:(g + 1) * 512],
                                             lhsT=w1f[:, c, :],
                                             rhs=X2T[:, c, g * 512:(g + 1) * 512],
                                             start=(c == 0), stop=(c == NC_DCH - 1))
                    nc.scalar.activation(out=h1[:, f, :], in_=ps, func=Relu,
                                         bias=b1_sb[:, f:f + 1], scale=1.0)
                for dd in range(8):
                    w2d = w2s.tile([P, 32, P], BF16, tag="w2d")
                    nc.sync.dma_start(out=w2d, in_=w2t[dd])
                    ps = mmd.tile([P, TQ], F32, tag="mm")
                    for g in range(2):
                        for fc in range(32):
                            nc.tensor.matmul(ps[:, g * 512:(g + 1) * 512],
                                             lhsT=w2d[:, fc, :],
                                             rhs=h1[:, fc, g * 512:(g + 1) * 512],
                                             start=(fc == 0), stop=(fc == 31))
                    fsb = scr2.tile([P, TQ], BF16, tag="fsb")
                    nc.vector.tensor_scalar_add(fsb, ps, b2_sb[:, dd:dd + 1])
                    ft = trx.tile([P, NQ, P], BF16, tag="tr")
                    for kb in range(NQ):
                        nc.tensor.matmul(ft[:, kb, :],
                                         lhsT=fsb[:, kb * P:(kb + 1) * P],
                                         rhs=id_bf, is_transpose=True,
                                         start=(kb == 0), stop=(kb == NQ - 1),
                                         skip_group_check=True)
                    nc.vector.tensor_add(xv[:, :, dd * P:(dd + 1) * P],
                                         xv[:, :, dd * P:(dd + 1) * P], ft)
                for kb in range(NQ):
                    nc.sync.dma_start(out=out_d[kb * P:(kb + 1) * P, :], in_=xv[:, kb, :])

    _split_drain_waits(nc)
    return nc


def _split_drain_waits(nc):
    """This walrus build gives every instruction a single hardware wait slot
    (one EVENTS struct per 64B instruction). Tile emits multi-wait
    instructions; move the excess waits onto single-wait NoOps inserted just
    before, on the same engine — identical semantics in program order."""
    for fn in nc.m.functions:
        for blk in fn.blocks:
            insts = blk.instructions
            i = 0
            while i < len(insts):
                inst = insts[i]
                si = inst.sync_info
                if si is not None and len(si.on_wait) > 1:
                    waits = list(si.on_wait)
                    inst.sync_info = mybir.SyncInfo(on_wait=[waits[-1]],
                                                    on_update=list(si.on_update))
                    for w in waits[:-1]:
                        nop = mybir.InstNoOp(name=nc.get_next_instruction_name(),
                                             ins=[], outs=[])
                        nop.engine = inst.engine
                        nop.sync_info = mybir.SyncInfo(on_wait=[w], on_update=[])
                        nc.register_instruction(nop, overwrite=True)
                        insts.insert(i, nop)
                        i += 1
                i += 1


def _prep_inputs(inputs, Wq, Wk, Wv, W1, b1, W2, b2, g1, be1, g2, be2,
                 apply_g1, apply_g2):
    bf = ml_dtypes.bfloat16
    f32 = np.float32
    inputs = np.ascontiguousarray(np.asarray(inputs, f32))
    wq_f = np.asarray(Wq, f32).transpose(1, 0, 2).reshape(D, D)
    wk_f = np.asarray(Wk, f32).transpose(1, 0, 2).reshape(D, D)
    wv_f = np.asarray(Wv, f32).transpose(1, 0, 2).reshape(D, D)

    def pair_tiles(w):  # [D, D] -> [8, 128, 8, 128] (pair, p, chunk, col)
        return np.ascontiguousarray(
            w.reshape(NC_DCH, P, 8, P).transpose(2, 1, 0, 3).astype(bf))

    wq_t, wk_t, wv_t = pair_tiles(wq_f), pair_tiles(wk_f), pair_tiles(wv_f)
    w1_t = np.ascontiguousarray(
        np.asarray(W1, f32).reshape(NC_DCH, P, 32, P).transpose(2, 1, 0, 3).astype(bf))
    w2_t = np.ascontiguousarray(
        np.asarray(W2, f32).reshape(32, P, 8, P).transpose(2, 1, 0, 3).astype(bf))
    b1_t = np.ascontiguousarray(np.asarray(b1, f32).reshape(32, P).T)
    b2_t = np.ascontiguousarray(np.asarray(b2, f32).reshape(8, P).T)

    cc, kk = np.meshgrid(np.arange(P), np.arange(64), indexing="ij")
    in_maps = []
    for c in range(8):
        b, o = divmod(c, 2)
        xb_c = inputs[b]
        xq_c = np.ascontiguousarray(xb_c[o::2, :])
        mask = np.ascontiguousarray((cc <= 2 * kk + o).astype(f32).astype(bf))
        m = {"xb": xb_c, "xq": xq_c, "wq": wq_t, "wk": wk_t, "wv": wv_t,
             "w1t": w1_t, "w2t": w2_t, "b1t": b1_t, "b2t": b2_t, "maskt": mask}
        if apply_g1:
            m["g1v"] = np.asarray(g1, f32)
            m["be1v"] = np.asarray(be1, f32)
        if apply_g2:
            m["g2v"] = np.asarray(g2, f32)
            m["be2v"] = np.asarray(be2, f32)
        in_maps.append(m)
    return in_maps


def _run(inputs, Wq, Wk, Wv, W1, b1, W2, b2, g1, be1, g2, be2, **spmd_kwargs):
    apply_g1 = not (np.all(np.asarray(g1) == 1.0) and np.all(np.asarray(be1) == 0.0))
    apply_g2 = not (np.all(np.asarray(g2) == 1.0) and np.all(np.asarray(be2) == 0.0))
    nc = build_program(apply_g1, apply_g2)
    in_maps = _prep_inputs(inputs, Wq, Wk, Wv, W1, b1, W2, b2, g1, be1, g2, be2,
                           apply_g1, apply_g2)
    res = run_bass_kernel_spmd(nc, in_maps, list(range(8)), **spmd_kwargs)
    out = np.empty((B, T, D), np.float32)
    for c in range(8):
        b, o = divmod(c, 2)
        out[b, o::2, :] = res.results[c]["out"]
    return out, res


def kernel(inputs, Wq, Wk, Wv, W1, b1, W2, b2, g1, be1, g2, be2):
    out, _ = _run(inputs, Wq, Wk, Wv, W1, b1, W2, b2, g1, be1, g2, be2)
    return out


# revision 18
# speedup vs baseline: 1.3170x; 1.0142x over previous
"""GPT block (LN -> causal MHA -> residual -> LN -> MLP -> residual) on 8 trn2 cores.

Sharding: core c = (batch b = c//2, parity o = c%2). Each core owns the
interleaved tokens o::2 of its batch. K/V are computed redundantly by the two
cores of a batch; attention rows and the MLP are token-parallel. No cross-core
communication: outputs are scattered back on the host.

All matmuls run in bf16 (fp32 PSUM accumulation); layernorm/softmax
normalization stats stay fp32. Structured to keep the PE array dense:
- scores for the two heads of a pair are issued interleaved at base
  partitions 0/64 so they run concurrently in disjoint row groups;
- exp runs on the scalar engine from dedicated score PSUM windows so its
  drain never blocks projection matmuls (separate PSUM pools);
- attn@V accumulates 512-query groups with causal suffix matmuls;
- all transposes are bf16 (1 cycle/col); LN normalization is applied by the
  scalar engine (Identity with per-partition scale/bias).
"""

import sys

if "/opt/trn_rl_repo" not in sys.path:
    sys.path.insert(0, "/opt/trn_rl_repo")

import numpy as np
import ml_dtypes

import concourse.bass as bass
import concourse.tile as tile
from concourse import mybir
from concourse.bass_utils import run_bass_kernel_spmd
from concourse.masks import make_identity

B, T, D, H, HD = 4, 2048, 1024, 16, 64
FF = 4 * D
P = 128
NB = T // P        # 16 key blocks
TQ = T // 2        # 1024 query tokens per core
NQ = TQ // P       # 8 query blocks per core
NC_DCH = D // P    # 8 contraction chunks over D
EPS = 1e-5
F32 = mybir.dt.float32
BF16 = mybir.dt.bfloat16
PT_TOTAL = sum(TQ - 64 * j for j in range(NB))  # 8704 = 17 * 512
NWIN = PT_TOTAL // 512  # 17 score windows per head

Exp = mybir.ActivationFunctionType.Exp
Relu = mybir.ActivationFunctionType.Relu
Sqrt = mybir.ActivationFunctionType.Sqrt
Ident = mybir.ActivationFunctionType.Identity
MUL = mybir.AluOpType.mult
ADD = mybir.AluOpType.add


def _pt_off(j):
    return 1024 * j - 32 * j * (j - 1)


def _score_chunks():
    """Flat causal score stream split at key-block and 512-col boundaries.
    Returns [(j, pos, w, off)] with off the packed pt column."""
    chunks = []
    off = 0
    for j in range(NB):
        slen = TQ - 64 * j
        pos = 0
        while pos < slen:
            w = min(512 - (off % 512), slen - pos)
            chunks.append((j, pos, w, off))
            pos += w
            off += w
    assert off == PT_TOTAL
    return chunks


def _layernorm(nc, lnp, src, dst_bf, eps_sb, gtile, btile, on_act=True):
    """LN over rows of src [P, D] f32 -> dst_bf [P, D] bf16. The normalize
    apply runs on the scalar engine (Identity(x*rstd - mu*rstd)) or on the
    vector engine (tensor_scalar) so callers can balance the two."""
    stats = lnp.tile([P, 2, 6], F32, tag="stats")
    for s in range(2):
        nc.vector.bn_stats(out=stats[:, s, :], in_=src[:, s * 512:(s + 1) * 512])
    mv = lnp.tile([P, 2], F32, tag="mv")
    nc.vector.bn_aggr(out=mv, in_=stats)
    rstd = lnp.tile([P, 1], F32, tag="rstd")
    nc.scalar.activation(out=rstd, in_=mv[:, 1:2], func=Sqrt, bias=eps_sb, scale=1.0)
    nc.vector.reciprocal(out=rstd, in_=rstd)
    if on_act:
        nmu = lnp.tile([P, 1], F32, tag="nmu")
        nc.vector.tensor_scalar(out=nmu, in0=mv[:, 0:1], scalar1=rstd, scalar2=-1.0,
                                op0=MUL, op1=MUL)
        nc.scalar.activation(out=dst_bf, in_=src, func=Ident, bias=nmu, scale=rstd)
    else:
        nc.vector.tensor_scalar(out=dst_bf, in0=src, scalar1=mv[:, 0:1],
                                scalar2=rstd, op0=mybir.AluOpType.subtract,
                                op1=MUL)
    if gtile is not None:
        nc.vector.tensor_mul(dst_bf, dst_bf, gtile)
    if btile is not None:
        nc.vector.tensor_add(dst_bf, dst_bf, btile)


def build_program(apply_g1=False, apply_g2=False):
    nc = bass.Bass()
    xb = nc.declare_dram_parameter("xb", [T, D], F32, isOutput=False)
    xq = nc.declare_dram_parameter("xq", [TQ, D], F32, isOutput=False)
    wq = nc.declare_dram_parameter("wq", [8, P, NC_DCH, P], BF16, isOutput=False)
    wk = nc.declare_dram_parameter("wk", [8, P, NC_DCH, P], BF16, isOutput=False)
    wv = nc.declare_dram_parameter("wv", [8, P, NC_DCH, P], BF16, isOutput=False)
    w1t = nc.declare_dram_parameter("w1t", [32, P, NC_DCH, P], BF16, isOutput=False)
    w2t = nc.declare_dram_parameter("w2t", [8, P, 32, P], BF16, isOutput=False)
    b1t = nc.declare_dram_parameter("b1t", [P, 32], F32, isOutput=False)
    b2t = nc.declare_dram_parameter("b2t", [P, 8], F32, isOutput=False)
    maskt = nc.declare_dram_parameter("maskt", [P, 64], BF16, isOutput=False)
    gb = {}
    if apply_g1:
        gb["g1"] = nc.declare_dram_parameter("g1v", [D], F32, isOutput=False)
        gb["be1"] = nc.declare_dram_parameter("be1v", [D], F32, isOutput=False)
    if apply_g2:
        gb["g2"] = nc.declare_dram_parameter("g2v", [D], F32, isOutput=False)
        gb["be2"] = nc.declare_dram_parameter("be2v", [D], F32, isOutput=False)
    out_d = nc.declare_dram_parameter("out", [TQ, D], F32, isOutput=True)

    chunks = _score_chunks()
    win_chunks = [[] for _ in range(NWIN)]
    for c in chunks:
        win_chunks[c[3] // 512].append(c)
    win_masks = [[] for _ in range(NWIN)]
    for j in range(NB):
        off = _pt_off(j)
        win_masks[off // 512].append(off)

    with tile.TileContext(nc) as tc:
        with tc.tile_pool(name="consts", bufs=1) as consts, \
             tc.tile_pool(name="big", bufs=1) as big, \
             tc.tile_pool(name="trx", bufs=2, space="PSUM") as trx:
            id_bf = consts.tile([P, P], BF16)
            make_identity(nc, id_bf)
            eps_sb = consts.tile([P, 1], F32)
            nc.vector.memset(eps_sb, EPS)
            mask_sb = consts.tile([P, 64], BF16)
            nc.sync.dma_start(out=mask_sb, in_=maskt[:, :])
            b1_sb = consts.tile([P, 32], F32)
            nc.sync.dma_start(out=b1_sb, in_=b1t[:, :])
            b2_sb = consts.tile([P, 8], F32)
            nc.sync.dma_start(out=b2_sb, in_=b2t[:, :])

            def bcast(name):
                t = consts.tile([P, D], F32, tag=f"bc_{name}")
                src = gb[name]
                ap = bass.AP(tensor=src.tensor if hasattr(src, "tensor") else src[:].tensor,
                             offset=src[:].offset, ap=[[0, P]] + list(src[:].ap))
                nc.sync.dma_start(out=t, in_=ap)
                return t

            g1_t = bcast("g1") if apply_g1 else None
            be1_t = bcast("be1") if apply_g1 else None
            g2_t = bcast("g2") if apply_g2 else None
            be2_t = bcast("be2") if apply_g2 else None

            XT = big.tile([P, NC_DCH, T], BF16)    # LN1(xb)^T
            XQT = big.tile([P, NC_DCH, TQ], BF16)  # strided query columns of XT
            xv = big.tile([P, NQ, D], F32)         # residual stream, my tokens

            # ---- Phase A: layernorm1 + transposes ----
            with tc.tile_pool(name="lnp", bufs=4) as lnp, \
                 tc.tile_pool(name="lnsrc", bufs=4) as lnsrc:
                for blk in range(NB):
                    x_t = lnsrc.tile([P, D], F32, tag="xsrc")
                    nc.sync.dma_start(out=x_t, in_=xb[blk * P:(blk + 1) * P, :])
                    xn = lnp.tile([P, D], BF16, tag="xn")
                    _layernorm(nc, lnp, x_t, xn, eps_sb, g1_t, be1_t,
                               on_act=(blk % 2 == 0))
                    lt = trx.tile([P, NC_DCH, P], BF16, tag="tr")
                    for c in range(NC_DCH):
                        nc.tensor.matmul(lt[:, c, :], lhsT=xn[:, c * P:(c + 1) * P],
                                         rhs=id_bf, is_transpose=True,
                                         start=(c == 0), stop=(c == NC_DCH - 1),
                                         skip_group_check=True)
                    if blk % 2 == 0:
                        nc.vector.tensor_copy(XT[:, :, blk * P:(blk + 1) * P], lt)
                    else:
                        nc.scalar.copy(XT[:, :, blk * P:(blk + 1) * P], lt)
                for kb in range(NQ):
                    nc.sync.dma_start(out=xv[:, kb, :], in_=xq[kb * P:(kb + 1) * P, :])
                    xnq = lnp.tile([P, D], BF16, tag="xn")
                    _layernorm(nc, lnp, xv[:, kb, :], xnq, eps_sb, g1_t, be1_t,
                               on_act=(kb % 2 == 0))
                    ltq = trx.tile([P, NC_DCH, P], BF16, tag="tr")
                    for c in range(NC_DCH):
                        nc.tensor.matmul(ltq[:, c, :], lhsT=xnq[:, c * P:(c + 1) * P],
                                         rhs=id_bf, is_transpose=True,
                                         start=(c == 0), stop=(c == NC_DCH - 1),
                                         skip_group_check=True)
                    if kb % 2 == 0:
                        nc.vector.tensor_copy(XQT[:, :, kb * P:(kb + 1) * P], ltq)
                    else:
                        nc.scalar.copy(XQT[:, :, kb * P:(kb + 1) * P], ltq)

            # ---- Phase B/C: per head-pair projections + attention ----
            with tc.tile_pool(name="wp", bufs=2) as wp, \
                 tc.tile_pool(name="ap", bufs=2) as apool, \
                 tc.tile_pool(name="ptp", bufs=1) as ptp, \
                 tc.tile_pool(name="scr", bufs=4) as scr, \
                 tc.tile_pool(name="pp", bufs=2, space="PSUM") as pp, \
                 tc.tile_pool(name="sw", bufs=2, space="PSUM") as sw, \
                 tc.tile_pool(name="avp", bufs=2, space="PSUM") as avp:
                for pr in range(8):
                    wq_p = wp.tile([P, NC_DCH, P], BF16, tag="wq")
                    nc.sync.dma_start(out=wq_p, in_=wq[pr])
                    wk_p = wp.tile([P, NC_DCH, P], BF16, tag="wk")
                    nc.sync.dma_start(out=wk_p, in_=wk[pr])
                    wv_p = wp.tile([P, NC_DCH, P], BF16, tag="wv")
                    nc.sync.dma_start(out=wv_p, in_=wv[pr])

                    KT_p = apool.tile([P, T], BF16, tag="kt")
                    for tg in range(4):
                        ps = pp.tile([P, 512], F32, tag="mm")
                        for c in range(NC_DCH):
                            nc.tensor.matmul(ps, lhsT=wk_p[:, c, :],
                                             rhs=XT[:, c, tg * 512:(tg + 1) * 512],
                                             start=(c == 0), stop=(c == NC_DCH - 1))
                        nc.vector.tensor_copy(KT_p[:, tg * 512:(tg + 1) * 512], ps)

                    Vaug_p = apool.tile([P, 2, NB, 65], BF16, tag="vaug")
                    nc.vector.memset(Vaug_p[:, :, :, 64:65], 1.0)
                    for tg in range(4):
                        ps = pp.tile([P, 512], F32, tag="mm")
                        for c in range(NC_DCH):
                            nc.tensor.matmul(ps, lhsT=wv_p[:, c, :],
                                             rhs=XT[:, c, tg * 512:(tg + 1) * 512],
                                             start=(c == 0), stop=(c == NC_DCH - 1))
                        vt_sb = scr.tile([P, 512], BF16, tag="vt")
                        nc.vector.tensor_copy(vt_sb, ps)
                        for hh in range(2):
                            # one accumulation group per bank, uniform base
                            # partition within the group (mixed bases wedge
                            # the PE)
                            tb = trx.tile([P, 4, 64], BF16, tag="tr")
                            for s in range(4):
                                nc.tensor.matmul(
                                    tb[:, s, :],
                                    lhsT=vt_sb[hh * 64:(hh + 1) * 64, s * P:(s + 1) * P],
                                    rhs=id_bf[hh * 64:(hh + 1) * 64, hh * 64:hh * 64 + 64],
                                    is_transpose=True,
                                    start=(s == 0), stop=(s == 3),
                                    skip_group_check=True)
                            nc.vector.tensor_copy(
                                Vaug_p[:, hh, tg * 4:(tg + 1) * 4, 0:64], tb)

                    QT_p = apool.tile([P, TQ], BF16, tag="qt")
                    for tg in range(2):
                        ps = pp.tile([P, 512], F32, tag="mm")
                        for c in range(NC_DCH):
                            nc.tensor.matmul(ps, lhsT=wq_p[:, c, :],
                                             rhs=XQT[:, c, tg * 512:(tg + 1) * 512],
                                             start=(c == 0), stop=(c == NC_DCH - 1))
                        nc.vector.tensor_copy(QT_p[:, tg * 512:(tg + 1) * 512], ps)

                    # scores: both heads interleaved (row groups 0-63 / 64-127)
                    pt0 = ptp.tile([P, PT_TOTAL], BF16, tag="pt0")
                    pt1 = ptp.tile([P, PT_TOTAL], BF16, tag="pt1")
                    pts = [pt0, pt1]
                    for w in range(NWIN):
                        win0 = sw.tile([P, 512], F32, tag="sw")
                        win1 = sw.tile([P, 512], F32, tag="sw")
                        wins = [win0, win1]
                        ncw = len(win_chunks[w])
                        for ci, (j, pos, wd, off) in enumerate(win_chunks[w]):
                            wcol = off % 512
                            for hh in range(2):
                                hs = slice(hh * 64, (hh + 1) * 64)
                                nc.tensor.matmul(
                                    wins[hh][:, wcol:wcol + wd],
                                    lhsT=KT_p[hs, j * P:(j + 1) * P],
                                    rhs=QT_p[hs, 64 * j + pos: 64 * j + pos + wd],
                                    start=(ci == 0), stop=(ci == ncw - 1),
                                    skip_group_check=True)
                        for hh in range(2):
                            nc.scalar.activation(
                                out=pts[hh][:, w * 512:(w + 1) * 512],
                                in_=wins[hh], func=Exp, scale=0.125)
                        for off in win_masks[w]:
                            for hh in range(2):
                                nc.vector.tensor_mul(pts[hh][:, off:off + 64],
                                                     pts[hh][:, off:off + 64],
                                                     mask_sb)

                    # attn @ V: 512-query groups, causal suffix accumulation
                    for hh in range(2):
                        h = pr * 2 + hh
                        pt = pts[hh]
                        for g in range(2):
                            ot = avp.tile([65, 512], F32, tag="av")
                            jmax = 8 * (g + 1)
                            for j in range(jmax):
                                qlo = max(512 * g, 64 * j)
                                wd = 512 * (g + 1) - qlo
                                nc.tensor.matmul(
                                    ot[:, qlo - 512 * g: 512],
                                    lhsT=Vaug_p[:, hh, j, :],
                                    rhs=pt[:, _pt_off(j) + qlo - 64 * j:
                                           _pt_off(j) + qlo - 64 * j + wd],
                                    start=(j == 0), stop=(j == jmax - 1))
                            ob = scr.tile([65, 512], BF16, tag="ob")
                            nc.vector.tensor_copy(ob, ot)
                            tp = trx.tile([P, 4, 96], BF16, tag="tr")
                            for s in range(4):
                                nc.tensor.matmul(tp[:, s, 0:65],
                                                 lhsT=ob[:, s * P:(s + 1) * P],
                                                 rhs=id_bf[0:65, 0:65],
                                                 is_transpose=True,
                                                 start=(s == 0), stop=(s == 3),
                                                 skip_group_check=True)
                            rd = scr.tile([P, 4], F32, tag="rd")
                            nc.vector.reciprocal(rd, tp[:, :, 64])
                            for s in range(4):
                                kb = 4 * g + s
                                nc.vector.scalar_tensor_tensor(
                                    out=xv[:, kb, h * 64:(h + 1) * 64],
                                    in0=tp[:, s, 0:64],
                                    scalar=rd[:, s:s + 1],
                                    in1=xv[:, kb, h * 64:(h + 1) * 64],
                                    op0=MUL, op1=ADD)

            # ---- Phase D: LN2 + MLP + residual over all 1024 tokens ----
            with tc.tile_pool(name="x2tp", bufs=1) as x2tp, \
                 tc.tile_pool(name="h1p", bufs=1) as h1p, \
                 tc.tile_pool(name="w1s", bufs=3) as w1s, \
                 tc.tile_pool(name="w2s", bufs=2) as w2s, \
                 tc.tile_pool(name="lnp2", bufs=2) as lnp2, \
                 tc.tile_pool(name="scr2", bufs=3) as scr2, \
                 tc.tile_pool(name="mmd", bufs=3, space="PSUM") as mmd:
                X2T = x2tp.tile([P, NC_DCH, TQ], BF16, tag="x2t")
                for kb in range(NQ):
                    xn2 = lnp2.tile([P, D], BF16, tag="xn2")
                    _layernorm(nc, lnp2, xv[:, kb, :], xn2, eps_sb, g2_t, be2_t)
                    lt = trx.tile([P, NC_DCH, P], BF16, tag="tr")
                    for c in range(NC_DCH):
                        nc.tensor.matmul(lt[:, c, :], lhsT=xn2[:, c * P:(c + 1) * P],
                                         rhs=id_bf, is_transpose=True,
                                         start=(c == 0), stop=(c == NC_DCH - 1),
                                         skip_group_check=True)
                    nc.vector.tensor_copy(X2T[:, :, kb * P:(kb + 1) * P], lt)
                h1 = h1p.tile([P, 32, TQ], BF16, tag="h1")
                for f in range(32):
                    w1f = w1s.tile([P, NC_DCH, P], BF16, tag="w1f")
                    nc.sync.dma_start(out=w1f, in_=w1t[f])
                    ps = mmd.tile([P, TQ], F32, tag="mm")
                    for g in range(2):
                        for c in range(NC_DCH):
                            nc.tensor.matmul(ps[:, g * 512:(g + 1) * 512],
                                             lhsT=w1f[:, c, :],
                                             rhs=X2T[:, c, g * 512:(g + 1) * 512],
                                             start=(c == 0), stop=(c == NC_DCH - 1))
                    nc.scalar.activation(out=h1[:, f, :], in_=ps, func=Relu,
                                         bias=b1_sb[:, f:f + 1], scale=1.0)
                for dd in range(8):
                    w2d = w2s.tile([P, 32, P], BF16, tag="w2d")
                    nc.sync.dma_start(out=w2d, in_=w2t[dd])
                    ps = mmd.tile([P, TQ], F32, tag="mm")
                    for g in range(2):
                        for fc in range(32):
                            nc.tensor.matmul(ps[:, g * 512:(g + 1) * 512],
                                             lhsT=w2d[:, fc, :],
                                             rhs=h1[:, fc, g * 512:(g + 1) * 512],
                                             start=(fc == 0), stop=(fc == 31))
                    fsb = scr2.tile([P, TQ], BF16, tag="fsb")
                    nc.vector.tensor_scalar_add(fsb, ps, b2_sb[:, dd:dd + 1])
                    ft = trx.tile([P, NQ, P], BF16, tag="tr")
                    for kb in range(NQ):
                        nc.tensor.matmul(ft[:, kb, :],
                                         lhsT=fsb[:, kb * P:(kb + 1) * P],
                                         rhs=id_bf, is_transpose=True,
                                         start=(kb == 0), stop=(kb == NQ - 1),
                                         skip_group_check=True)
                    nc.vector.tensor_add(xv[:, :, dd * P:(dd + 1) * P],
                                         xv[:, :, dd * P:(dd + 1) * P], ft)
                    for kb in range(NQ):
                        nc.sync.dma_start(
                            out=out_d[kb * P:(kb + 1) * P, dd * P:(dd + 1) * P],
                            in_=xv[:, kb, dd * P:(dd + 1) * P])

    _split_drain_waits(nc)
    return nc


def _split_drain_waits(nc):
    """This walrus build gives every instruction a single hardware wait slot
    (one EVENTS struct per 64B instruction). Tile emits multi-wait
    instructions; move the excess waits onto single-wait NoOps inserted just
    before, on the same engine — identical semantics in program order."""
    for fn in nc.m.functions:
        for blk in fn.blocks:
            insts = blk.instructions
            i = 0
            while i < len(insts):
                inst = insts[i]
                si = inst.sync_info
                if si is not None and len(si.on_wait) > 1:
                    waits = list(si.on_wait)
                    inst.sync_info = mybir.SyncInfo(on_wait=[waits[-1]],
                                                    on_update=list(si.on_update))
                    for w in waits[:-1]:
                        nop = mybir.InstNoOp(name=nc.get_next_instruction_name(),
                                             ins=[], outs=[])
                        nop.engine = inst.engine
                        nop.sync_info = mybir.SyncInfo(on_wait=[w], on_update=[])
                        nc.register_instruction(nop, overwrite=True)
                        insts.insert(i, nop)
                        i += 1
                i += 1


def _prep_inputs(inputs, Wq, Wk, Wv, W1, b1, W2, b2, g1, be1, g2, be2,
                 apply_g1, apply_g2):
    bf = ml_dtypes.bfloat16
    f32 = np.float32
    inputs = np.ascontiguousarray(np.asarray(inputs, f32))
    wq_f = np.asarray(Wq, f32).transpose(1, 0, 2).reshape(D, D)
    wk_f = np.asarray(Wk, f32).transpose(1, 0, 2).reshape(D, D)
    wv_f = np.asarray(Wv, f32).transpose(1, 0, 2).reshape(D, D)

    def pair_tiles(w):  # [D, D] -> [8, 128, 8, 128] (pair, p, chunk, col)
        return np.ascontiguousarray(
            w.reshape(NC_DCH, P, 8, P).transpose(2, 1, 0, 3).astype(bf))

    wq_t, wk_t, wv_t = pair_tiles(wq_f), pair_tiles(wk_f), pair_tiles(wv_f)
    w1_t = np.ascontiguousarray(
        np.asarray(W1, f32).reshape(NC_DCH, P, 32, P).transpose(2, 1, 0, 3).astype(bf))
    w2_t = np.ascontiguousarray(
        np.asarray(W2, f32).reshape(32, P, 8, P).transpose(2, 1, 0, 3).astype(bf))
    b1_t = np.ascontiguousarray(np.asarray(b1, f32).reshape(32, P).T)
    b2_t = np.ascontiguousarray(np.asarray(b2, f32).reshape(8, P).T)

    cc, kk = np.meshgrid(np.arange(P), np.arange(64), indexing="ij")
    in_maps = []
    for c in range(8):
        b, o = divmod(c, 2)
        xb_c = inputs[b]
        xq_c = np.ascontiguousarray(xb_c[o::2, :])
        mask = np.ascontiguousarray((cc <= 2 * kk + o).astype(f32).astype(bf))
        m = {"xb": xb_c, "xq": xq_c, "wq": wq_t, "wk": wk_t, "wv": wv_t,
             "w1t": w1_t, "w2t": w2_t, "b1t": b1_t, "b2t": b2_t, "maskt": mask}
        if apply_g1:
            m["g1v"] = np.asarray(g1, f32)
            m["be1v"] = np.asarray(be1, f32)
        if apply_g2:
            m["g2v"] = np.asarray(g2, f32)
            m["be2v"] = np.asarray(be2, f32)
        in_maps.append(m)
    return in_maps


def _run(inputs, Wq, Wk, Wv, W1, b1, W2, b2, g1, be1, g2, be2, **spmd_kwargs):
    apply_g1 = not (np.all(np.asarray(g1) == 1.0) and np.all(np.asarray(be1) == 0.0))
    apply_g2 = not (np.all(np.asarray(g2) == 1.0) and np.all(np.asarray(be2) == 0.0))
    nc = build_program(apply_g1, apply_g2)
    in_maps = _prep_inputs(inputs, Wq, Wk, Wv, W1, b1, W2, b2, g1, be1, g2, be2,
                           apply_g1, apply_g2)
    res = run_bass_kernel_spmd(nc, in_maps, list(range(8)), **spmd_kwargs)
    out = np.empty((B, T, D), np.float32)
    for c in range(8):
        b, o = divmod(c, 2)
        out[b, o::2, :] = res.results[c]["out"]
    return out, res


def kernel(inputs, Wq, Wk, Wv, W1, b1, W2, b2, g1, be1, g2, be2):
    out, _ = _run(inputs, Wq, Wk, Wv, W1, b1, W2, b2, g1, be1, g2, be2)
    return out


# revision 19
# speedup vs baseline: 1.3364x; 1.0147x over previous
"""GPT block (LN -> causal MHA -> residual -> LN -> MLP -> residual) on 8 trn2 cores.

Sharding: core c = (batch b = c//2, parity o = c%2). Each core owns the
interleaved tokens o::2 of its batch. K/V are computed redundantly by the two
cores of a batch; attention rows and the MLP are token-parallel. No cross-core
communication: outputs are scattered back on the host.

All matmuls run in bf16 (fp32 PSUM accumulation); layernorm/softmax
normalization stats stay fp32. Structured to keep the PE array dense:
- scores for the two heads of a pair are issued interleaved at base
  partitions 0/64 so they run concurrently in disjoint row groups;
- exp runs on the scalar engine from dedicated score PSUM windows so its
  drain never blocks projection matmuls (separate PSUM pools);
- attn@V accumulates 512-query groups with causal suffix matmuls;
- all transposes are bf16 (1 cycle/col); LN normalization is applied by the
  scalar engine (Identity with per-partition scale/bias).
"""

import sys

if "/opt/trn_rl_repo" not in sys.path:
    sys.path.insert(0, "/opt/trn_rl_repo")

import numpy as np
import ml_dtypes

import concourse.bass as bass
import concourse.tile as tile
from concourse import mybir
from concourse.bass_utils import run_bass_kernel_spmd
from concourse.masks import make_identity

B, T, D, H, HD = 4, 2048, 1024, 16, 64
FF = 4 * D
P = 128
NB = T // P        # 16 key blocks
TQ = T // 2        # 1024 query tokens per core
NQ = TQ // P       # 8 query blocks per core
NC_DCH = D // P    # 8 contraction chunks over D
EPS = 1e-5
F32 = mybir.dt.float32
BF16 = mybir.dt.bfloat16
PT_TOTAL = sum(TQ - 64 * j for j in range(NB))  # 8704 = 17 * 512
NWIN = PT_TOTAL // 512  # 17 score windows per head

Exp = mybir.ActivationFunctionType.Exp
Relu = mybir.ActivationFunctionType.Relu
Sqrt = mybir.ActivationFunctionType.Sqrt
Ident = mybir.ActivationFunctionType.Identity
MUL = mybir.AluOpType.mult
ADD = mybir.AluOpType.add


def _pt_off(j):
    return 1024 * j - 32 * j * (j - 1)


def _score_chunks():
    """Flat causal score stream split at key-block and 512-col boundaries.
    Returns [(j, pos, w, off)] with off the packed pt column."""
    chunks = []
    off = 0
    for j in range(NB):
        slen = TQ - 64 * j
        pos = 0
        while pos < slen:
            w = min(512 - (off % 512), slen - pos)
            chunks.append((j, pos, w, off))
            pos += w
            off += w
    assert off == PT_TOTAL
    return chunks


def _layernorm(nc, lnp, src, dst_bf, eps_sb, gtile, btile, on_act=True):
    """LN over rows of src [P, D] f32 -> dst_bf [P, D] bf16. The normalize
    apply runs on the scalar engine (Identity(x*rstd - mu*rstd)) or on the
    vector engine (tensor_scalar) so callers can balance the two."""
    stats = lnp.tile([P, 2, 6], F32, tag="stats")
    for s in range(2):
        nc.vector.bn_stats(out=stats[:, s, :], in_=src[:, s * 512:(s + 1) * 512])
    mv = lnp.tile([P, 2], F32, tag="mv")
    nc.vector.bn_aggr(out=mv, in_=stats)
    rstd = lnp.tile([P, 1], F32, tag="rstd")
    nc.scalar.activation(out=rstd, in_=mv[:, 1:2], func=Sqrt, bias=eps_sb, scale=1.0)
    nc.vector.reciprocal(out=rstd, in_=rstd)
    if on_act:
        nmu = lnp.tile([P, 1], F32, tag="nmu")
        nc.vector.tensor_scalar(out=nmu, in0=mv[:, 0:1], scalar1=rstd, scalar2=-1.0,
                                op0=MUL, op1=MUL)
        nc.scalar.activation(out=dst_bf, in_=src, func=Ident, bias=nmu, scale=rstd)
    else:
        nc.vector.tensor_scalar(out=dst_bf, in0=src, scalar1=mv[:, 0:1],
                                scalar2=rstd, op0=mybir.AluOpType.subtract,
                                op1=MUL)
    if gtile is not None:
        nc.vector.tensor_mul(dst_bf, dst_bf, gtile)
    if btile is not None:
        nc.vector.tensor_add(dst_bf, dst_bf, btile)


def build_program(apply_g1=False, apply_g2=False):
    nc = bass.Bass()
    xb = nc.declare_dram_parameter("xb", [T, D], F32, isOutput=False)
    xq = nc.declare_dram_parameter("xq", [TQ, D], F32, isOutput=False)
    wq = nc.declare_dram_parameter("wq", [8, P, NC_DCH, P], BF16, isOutput=False)
    wk = nc.declare_dram_parameter("wk", [8, P, NC_DCH, P], BF16, isOutput=False)
    wv = nc.declare_dram_parameter("wv", [8, P, NC_DCH, P], BF16, isOutput=False)
    w1t = nc.declare_dram_parameter("w1t", [32, P, NC_DCH, P], BF16, isOutput=False)
    w2t = nc.declare_dram_parameter("w2t", [8, P, 32, P], BF16, isOutput=False)
    b1t = nc.declare_dram_parameter("b1t", [P, 32], F32, isOutput=False)
    b2t = nc.declare_dram_parameter("b2t", [P, 8], F32, isOutput=False)
    maskt = nc.declare_dram_parameter("maskt", [P, 64], BF16, isOutput=False)
    gb = {}
    if apply_g1:
        gb["g1"] = nc.declare_dram_parameter("g1v", [D], F32, isOutput=False)
        gb["be1"] = nc.declare_dram_parameter("be1v", [D], F32, isOutput=False)
    if apply_g2:
        gb["g2"] = nc.declare_dram_parameter("g2v", [D], F32, isOutput=False)
        gb["be2"] = nc.declare_dram_parameter("be2v", [D], F32, isOutput=False)
    out_d = nc.declare_dram_parameter("out", [TQ, D], F32, isOutput=True)

    chunks = _score_chunks()
    win_chunks = [[] for _ in range(NWIN)]
    for c in chunks:
        win_chunks[c[3] // 512].append(c)
    win_masks = [[] for _ in range(NWIN)]
    for j in range(NB):
        off = _pt_off(j)
        win_masks[off // 512].append(off)

    with tile.TileContext(nc) as tc:
        with tc.tile_pool(name="consts", bufs=1) as consts, \
             tc.tile_pool(name="big", bufs=1) as big, \
             tc.tile_pool(name="trx", bufs=2, space="PSUM") as trx:
            id_bf = consts.tile([P, P], BF16)
            make_identity(nc, id_bf)
            eps_sb = consts.tile([P, 1], F32)
            nc.vector.memset(eps_sb, EPS)
            mask_sb = consts.tile([P, 64], BF16)
            nc.sync.dma_start(out=mask_sb, in_=maskt[:, :])
            b1_sb = consts.tile([P, 32], F32)
            nc.sync.dma_start(out=b1_sb, in_=b1t[:, :])
            b2_sb = consts.tile([P, 8], F32)
            nc.sync.dma_start(out=b2_sb, in_=b2t[:, :])

            def bcast(name):
                t = consts.tile([P, D], F32, tag=f"bc_{name}")
                src = gb[name]
                ap = bass.AP(tensor=src.tensor if hasattr(src, "tensor") else src[:].tensor,
                             offset=src[:].offset, ap=[[0, P]] + list(src[:].ap))
                nc.sync.dma_start(out=t, in_=ap)
                return t

            g1_t = bcast("g1") if apply_g1 else None
            be1_t = bcast("be1") if apply_g1 else None
            g2_t = bcast("g2") if apply_g2 else None
            be2_t = bcast("be2") if apply_g2 else None

            XT = big.tile([P, NC_DCH, T], BF16)    # LN1(xb)^T
            XQT = big.tile([P, NC_DCH, TQ], BF16)  # strided query columns of XT
            xv = big.tile([P, NQ, D], F32)         # residual stream, my tokens

            # ---- Phase A: layernorm1 + transposes ----
            with tc.tile_pool(name="lnp", bufs=4) as lnp, \
                 tc.tile_pool(name="lnsrc", bufs=4) as lnsrc:
                for blk in range(NB):
                    x_t = lnsrc.tile([P, D], F32, tag="xsrc")
                    nc.sync.dma_start(out=x_t, in_=xb[blk * P:(blk + 1) * P, :])
                    xn = lnp.tile([P, D], BF16, tag="xn")
                    _layernorm(nc, lnp, x_t, xn, eps_sb, g1_t, be1_t,
                               on_act=(blk % 2 == 0))
                    lt = trx.tile([P, NC_DCH, P], BF16, tag="tr")
                    for c in range(NC_DCH):
                        nc.tensor.matmul(lt[:, c, :], lhsT=xn[:, c * P:(c + 1) * P],
                                         rhs=id_bf, is_transpose=True,
                                         start=(c == 0), stop=(c == NC_DCH - 1),
                                         skip_group_check=True)
                    if blk % 2 == 0:
                        nc.vector.tensor_copy(XT[:, :, blk * P:(blk + 1) * P], lt)
                    else:
                        nc.scalar.copy(XT[:, :, blk * P:(blk + 1) * P], lt)
                for kb in range(NQ):
                    nc.sync.dma_start(out=xv[:, kb, :], in_=xq[kb * P:(kb + 1) * P, :])
                    xnq = lnp.tile([P, D], BF16, tag="xn")
                    _layernorm(nc, lnp, xv[:, kb, :], xnq, eps_sb, g1_t, be1_t,
                               on_act=(kb % 2 == 0))
                    ltq = trx.tile([P, NC_DCH, P], BF16, tag="tr")
                    for c in range(NC_DCH):
                        nc.tensor.matmul(ltq[:, c, :], lhsT=xnq[:, c * P:(c + 1) * P],
                                         rhs=id_bf, is_transpose=True,
                                         start=(c == 0), stop=(c == NC_DCH - 1),
                                         skip_group_check=True)
                    if kb % 2 == 0:
                        nc.vector.tensor_copy(XQT[:, :, kb * P:(kb + 1) * P], ltq)
                    else:
                        nc.scalar.copy(XQT[:, :, kb * P:(kb + 1) * P], ltq)

            # ---- Phase B/C: per head-pair projections + attention ----
            with tc.tile_pool(name="wp", bufs=2) as wp, \
                 tc.tile_pool(name="ap", bufs=2) as apool, \
                 tc.tile_pool(name="ptp", bufs=1) as ptp, \
                 tc.tile_pool(name="scr", bufs=4) as scr, \
                 tc.tile_pool(name="pp", bufs=2, space="PSUM") as pp, \
                 tc.tile_pool(name="sw", bufs=3, space="PSUM") as sw, \
                 tc.tile_pool(name="avp", bufs=1, space="PSUM") as avp:
                for pr in range(8):
                    wq_p = wp.tile([P, NC_DCH, P], BF16, tag="wq")
                    nc.sync.dma_start(out=wq_p, in_=wq[pr])
                    wk_p = wp.tile([P, NC_DCH, P], BF16, tag="wk")
                    nc.sync.dma_start(out=wk_p, in_=wk[pr])
                    wv_p = wp.tile([P, NC_DCH, P], BF16, tag="wv")
                    nc.sync.dma_start(out=wv_p, in_=wv[pr])

                    KT_p = apool.tile([P, T], BF16, tag="kt")
                    for tg in range(4):
                        ps = pp.tile([P, 512], F32, tag="mm")
                        for c in range(NC_DCH):
                            nc.tensor.matmul(ps, lhsT=wk_p[:, c, :],
                                             rhs=XT[:, c, tg * 512:(tg + 1) * 512],
                                             start=(c == 0), stop=(c == NC_DCH - 1))
                        nc.vector.tensor_copy(KT_p[:, tg * 512:(tg + 1) * 512], ps)

                    Vaug_p = apool.tile([P, 2, NB, 65], BF16, tag="vaug")
                    nc.vector.memset(Vaug_p[:, :, :, 64:65], 1.0)
                    for tg in range(4):
                        ps = pp.tile([P, 512], F32, tag="mm")
                        for c in range(NC_DCH):
                            nc.tensor.matmul(ps, lhsT=wv_p[:, c, :],
                                             rhs=XT[:, c, tg * 512:(tg + 1) * 512],
                                             start=(c == 0), stop=(c == NC_DCH - 1))
                        vt_sb = scr.tile([P, 512], BF16, tag="vt")
                        nc.vector.tensor_copy(vt_sb, ps)
                        for hh in range(2):
                            # one accumulation group per bank, uniform base
                            # partition within the group (mixed bases wedge
                            # the PE)
                            tb = trx.tile([P, 4, 64], BF16, tag="tr")
                            for s in range(4):
                                nc.tensor.matmul(
                                    tb[:, s, :],
                                    lhsT=vt_sb[hh * 64:(hh + 1) * 64, s * P:(s + 1) * P],
                                    rhs=id_bf[hh * 64:(hh + 1) * 64, hh * 64:hh * 64 + 64],
                                    is_transpose=True,
                                    start=(s == 0), stop=(s == 3),
                                    skip_group_check=True)
                            nc.vector.tensor_copy(
                                Vaug_p[:, hh, tg * 4:(tg + 1) * 4, 0:64], tb)

                    QT_p = apool.tile([P, TQ], BF16, tag="qt")
                    for tg in range(2):
                        ps = pp.tile([P, 512], F32, tag="mm")
                        for c in range(NC_DCH):
                            nc.tensor.matmul(ps, lhsT=wq_p[:, c, :],
                                             rhs=XQT[:, c, tg * 512:(tg + 1) * 512],
                                             start=(c == 0), stop=(c == NC_DCH - 1))
                        nc.vector.tensor_copy(QT_p[:, tg * 512:(tg + 1) * 512], ps)

                    # scores: both heads interleaved (row groups 0-63 / 64-127)
                    pt0 = ptp.tile([P, PT_TOTAL], BF16, tag="pt0")
                    pt1 = ptp.tile([P, PT_TOTAL], BF16, tag="pt1")
                    pts = [pt0, pt1]
                    for w in range(NWIN):
                        win0 = sw.tile([P, 512], F32, tag="sw")
                        win1 = sw.tile([P, 512], F32, tag="sw")
                        wins = [win0, win1]
                        ncw = len(win_chunks[w])
                        for ci, (j, pos, wd, off) in enumerate(win_chunks[w]):
                            wcol = off % 512
                            for hh in range(2):
                                hs = slice(hh * 64, (hh + 1) * 64)
                                nc.tensor.matmul(
                                    wins[hh][:, wcol:wcol + wd],
                                    lhsT=KT_p[hs, j * P:(j + 1) * P],
                                    rhs=QT_p[hs, 64 * j + pos: 64 * j + pos + wd],
                                    start=(ci == 0), stop=(ci == ncw - 1),
                                    skip_group_check=True)
                        for hh in range(2):
                            nc.scalar.activation(
                                out=pts[hh][:, w * 512:(w + 1) * 512],
                                in_=wins[hh], func=Exp, scale=0.125)
                        for off in win_masks[w]:
                            for hh in range(2):
                                nc.vector.tensor_mul(pts[hh][:, off:off + 64],
                                                     pts[hh][:, off:off + 64],
                                                     mask_sb)

                    # attn @ V: 512-query groups, causal suffix accumulation
                    for hh in range(2):
                        h = pr * 2 + hh
                        pt = pts[hh]
                        for g in range(2):
                            ot = avp.tile([65, 512], F32, tag="av")
                            jmax = 8 * (g + 1)
                            for j in range(jmax):
                                qlo = max(512 * g, 64 * j)
                                wd = 512 * (g + 1) - qlo
                                nc.tensor.matmul(
                                    ot[:, qlo - 512 * g: 512],
                                    lhsT=Vaug_p[:, hh, j, :],
                                    rhs=pt[:, _pt_off(j) + qlo - 64 * j:
                                           _pt_off(j) + qlo - 64 * j + wd],
                                    start=(j == 0), stop=(j == jmax - 1))
                            ob = scr.tile([65, 512], BF16, tag="ob")
                            nc.vector.tensor_copy(ob, ot)
                            tp = trx.tile([P, 4, 96], BF16, tag="tr")
                            for s in range(4):
                                nc.tensor.matmul(tp[:, s, 0:65],
                                                 lhsT=ob[:, s * P:(s + 1) * P],
                                                 rhs=id_bf[0:65, 0:65],
                                                 is_transpose=True,
                                                 start=(s == 0), stop=(s == 3),
                                                 skip_group_check=True)
                            rd = scr.tile([P, 4], F32, tag="rd")
                            nc.vector.reciprocal(rd, tp[:, :, 64])
                            for s in range(4):
                                kb = 4 * g + s
                                nc.vector.scalar_tensor_tensor(
                                    out=xv[:, kb, h * 64:(h + 1) * 64],
                                    in0=tp[:, s, 0:64],
                                    scalar=rd[:, s:s + 1],
                                    in1=xv[:, kb, h * 64:(h + 1) * 64],
                                    op0=MUL, op1=ADD)

            # ---- Phase D: LN2 + MLP + residual over all 1024 tokens ----
            with tc.tile_pool(name="x2tp", bufs=1) as x2tp, \
                 tc.tile_pool(name="h1p", bufs=1) as h1p, \
                 tc.tile_pool(name="w1s", bufs=3) as w1s, \
                 tc.tile_pool(name="w2s", bufs=2) as w2s, \
                 tc.tile_pool(name="lnp2", bufs=2) as lnp2, \
                 tc.tile_pool(name="scr2", bufs=3) as scr2, \
                 tc.tile_pool(name="mmd", bufs=3, space="PSUM") as mmd:
                X2T = x2tp.tile([P, NC_DCH, TQ], BF16, tag="x2t")
                for kb in range(NQ):
                    xn2 = lnp2.tile([P, D], BF16, tag="xn2")
                    _layernorm(nc, lnp2, xv[:, kb, :], xn2, eps_sb, g2_t, be2_t)
                    lt = trx.tile([P, NC_DCH, P], BF16, tag="tr")
                    for c in range(NC_DCH):
                        nc.tensor.matmul(lt[:, c, :], lhsT=xn2[:, c * P:(c + 1) * P],
                                         rhs=id_bf, is_transpose=True,
                                         start=(c == 0), stop=(c == NC_DCH - 1),
                                         skip_group_check=True)
                    nc.vector.tensor_copy(X2T[:, :, kb * P:(kb + 1) * P], lt)
                h1 = h1p.tile([P, 32, TQ], BF16, tag="h1")
                for f in range(32):
                    w1f = w1s.tile([P, NC_DCH, P], BF16, tag="w1f")
                    nc.sync.dma_start(out=w1f, in_=w1t[f])
                    ps = mmd.tile([P, TQ], F32, tag="mm")
                    for g in range(2):
                        for c in range(NC_DCH):
                            nc.tensor.matmul(ps[:, g * 512:(g + 1) * 512],
                                             lhsT=w1f[:, c, :],
                                             rhs=X2T[:, c, g * 512:(g + 1) * 512],
                                             start=(c == 0), stop=(c == NC_DCH - 1))
                    nc.scalar.activation(out=h1[:, f, :], in_=ps, func=Relu,
                                         bias=b1_sb[:, f:f + 1], scale=1.0)
                for dd in range(8):
                    w2d = w2s.tile([P, 32, P], BF16, tag="w2d")
                    nc.sync.dma_start(out=w2d, in_=w2t[dd])
                    ps = mmd.tile([P, TQ], F32, tag="mm")
                    for g in range(2):
                        for fc in range(32):
                            nc.tensor.matmul(ps[:, g * 512:(g + 1) * 512],
                                             lhsT=w2d[:, fc, :],
                                             rhs=h1[:, fc, g * 512:(g + 1) * 512],
                                             start=(fc == 0), stop=(fc == 31))
                    fsb = scr2.tile([P, TQ], BF16, tag="fsb")
                    nc.vector.tensor_scalar_add(fsb, ps, b2_sb[:, dd:dd + 1])
                    ft = trx.tile([P, NQ, P], BF16, tag="tr")
                    for kb in range(NQ):
                        nc.tensor.matmul(ft[:, kb, :],
                                         lhsT=fsb[:, kb * P:(kb + 1) * P],
                                         rhs=id_bf, is_transpose=True,
                                         start=(kb == 0), stop=(kb == NQ - 1),
                                         skip_group_check=True)
                    nc.vector.tensor_add(xv[:, :, dd * P:(dd + 1) * P],
                                         xv[:, :, dd * P:(dd + 1) * P], ft)
                    for kb in range(NQ):
                        nc.sync.dma_start(
                            out=out_d[kb * P:(kb + 1) * P, dd * P:(dd + 1) * P],
                            in_=xv[:, kb, dd * P:(dd + 1) * P])

    _split_drain_waits(nc)
    return nc


def _split_drain_waits(nc):
    """This walrus build gives every instruction a single hardware wait slot
    (one EVENTS struct per 64B instruction). Tile emits multi-wait
    instructions; move the excess waits onto single-wait NoOps inserted just
    before, on the same engine — identical semantics in program order."""
    for fn in nc.m.functions:
        for blk in fn.blocks:
            insts = blk.instructions
            i = 0
            while i < len(insts):
                inst = insts[i]
                si = inst.sync_info
                if si is not None and len(si.on_wait) > 1:
                    waits = list(si.on_wait)
                    inst.sync_info = mybir.SyncInfo(on_wait=[waits[-1]],
                                                    on_update=list(si.on_update))
                    for w in waits[:-1]:
                        nop = mybir.InstNoOp(name=nc.get_next_instruction_name(),
                                             ins=[], outs=[])
                        nop.engine = inst.engine
                        nop.sync_info = mybir.SyncInfo(on_wait=[w], on_update=[])
                        nc.register_instruction(nop, overwrite=True)
                        insts.insert(i, nop)
                        i += 1
                i += 1


def _prep_inputs(inputs, Wq, Wk, Wv, W1, b1, W2, b2, g1, be1, g2, be2,
                 apply_g1, apply_g2):
    bf = ml_dtypes.bfloat16
    f32 = np.float32
    inputs = np.ascontiguousarray(np.asarray(inputs, f32))
    wq_f = np.asarray(Wq, f32).transpose(1, 0, 2).reshape(D, D)
    wk_f = np.asarray(Wk, f32).transpose(1, 0, 2).reshape(D, D)
    wv_f = np.asarray(Wv, f32).transpose(1, 0, 2).reshape(D, D)

    def pair_tiles(w):  # [D, D] -> [8, 128, 8, 128] (pair, p, chunk, col)
        return np.ascontiguousarray(
            w.reshape(NC_DCH, P, 8, P).transpose(2, 1, 0, 3).astype(bf))

    wq_t, wk_t, wv_t = pair_tiles(wq_f), pair_tiles(wk_f), pair_tiles(wv_f)
    w1_t = np.ascontiguousarray(
        np.asarray(W1, f32).reshape(NC_DCH, P, 32, P).transpose(2, 1, 0, 3).astype(bf))
    w2_t = np.ascontiguousarray(
        np.asarray(W2, f32).reshape(32, P, 8, P).transpose(2, 1, 0, 3).astype(bf))
    b1_t = np.ascontiguousarray(np.asarray(b1, f32).reshape(32, P).T)
    b2_t = np.ascontiguousarray(np.asarray(b2, f32).reshape(8, P).T)

    cc, kk = np.meshgrid(np.arange(P), np.arange(64), indexing="ij")
    in_maps = []
    for c in range(8):
        b, o = divmod(c, 2)
        xb_c = inputs[b]
        xq_c = np.ascontiguousarray(xb_c[o::2, :])
        mask = np.ascontiguousarray((cc <= 2 * kk + o).astype(f32).astype(bf))
        m = {"xb": xb_c, "xq": xq_c, "wq": wq_t, "wk": wk_t, "wv": wv_t,
             "w1t": w1_t, "w2t": w2_t, "b1t": b1_t, "b2t": b2_t, "maskt": mask}
        if apply_g1:
            m["g1v"] = np.asarray(g1, f32)
            m["be1v"] = np.asarray(be1, f32)
        if apply_g2:
            m["g2v"] = np.asarray(g2, f32)
            m["be2v"] = np.asarray(be2, f32)
        in_maps.append(m)
    return in_maps


def _run(inputs, Wq, Wk, Wv, W1, b1, W2, b2, g1, be1, g2, be2, **spmd_kwargs):
    apply_g1 = not (np.all(np.asarray(g1) == 1.0) and np.all(np.asarray(be1) == 0.0))
    apply_g2 = not (np.all(np.asarray(g2) == 1.0) and np.all(np.asarray(be2) == 0.0))
    nc = build_program(apply_g1, apply_g2)
    in_maps = _prep_inputs(inputs, Wq, Wk, Wv, W1, b1, W2, b2, g1, be1, g2, be2,
                           apply_g1, apply_g2)
    res = run_bass_kernel_spmd(nc, in_maps, list(range(8)), **spmd_kwargs)
    out = np.empty((B, T, D), np.float32)
    for c in range(8):
        b, o = divmod(c, 2)
        out[b, o::2, :] = res.results[c]["out"]
    return out, res


def kernel(inputs, Wq, Wk, Wv, W1, b1, W2, b2, g1, be1, g2, be2):
    out, _ = _run(inputs, Wq, Wk, Wv, W1, b1, W2, b2, g1, be1, g2, be2)
    return out
